# revision 1
# baseline (speedup 1.0000x reference)
"""DAGNN (GRU + 2xGAT + mean-pool + fc + log_softmax) on 8 TRN2 cores via Bass/Tile.

Sharding: nodes split evenly across cores for the GRU; edges sorted by dst and
split at dst boundaries for the GAT layers (each dst wholly on one core);
feature tables AllGathered so gathers by src are local; graph pooling partial
sums AllReduced.
"""
import sys
import numpy as np

sys.path.insert(0, "/opt/trn_rl_repo")

import concourse.bass as bass
import concourse.bacc as bacc
import concourse.mybir as mybir
import concourse.tile as tile
from concourse.masks import make_identity

F32 = mybir.dt.float32
I32 = mybir.dt.int32
AF = mybir.ActivationFunctionType
OP = mybir.AluOpType

SENT_BIG = 0  # sentinel table row holds a_src = -100 so exp(leaky(...)) ~ 0


class Cfg:
    def __init__(self, N, E, G, n_cores, gru_tile=512):
        self.N, self.E, self.G, self.P = N, E, G, n_cores
        self.T, self.D, self.H = 8, 128, 64
        self.C1, self.HEADS, self.C2 = 256, 4, 64
        assert N % n_cores == 0
        self.NPC = N // n_cores
        self.gru_tile = gru_tile
        self.NPC_PAD = -(-self.NPC // gru_tile) * gru_tile
        self.SENT1 = n_cores * self.NPC_PAD          # sentinel row in table1
        self.TBL1 = self.SENT1 + 128
        self.NB = None  # max blocks per core, set by host_prep
        # table1 row of node n
    def trow(self, n):
        return (n // self.NPC) * self.NPC_PAD + (n % self.NPC)


def host_prep(cfg, edge_index, batch):
    """Sort edges by dst, split across cores at dst boundaries, pack into
    1024-edge blocks (8 tiles x 128) whose dst span < 128 and which contain
    whole dst segments. Returns per-core index arrays + metadata."""
    N, E, P = cfg.N, cfg.E, cfg.P
    src = np.concatenate([np.asarray(edge_index[0], np.int64), np.arange(N, dtype=np.int64)])
    dst = np.concatenate([np.asarray(edge_index[1], np.int64), np.arange(N, dtype=np.int64)])
    order = np.argsort(dst, kind="stable")
    ss, dd = src[order], dst[order]
    Etot = ss.shape[0]

    bounds = [0]
    for k in range(1, P):
        pos = (k * Etot) // P
        while pos < Etot and dd[pos] == dd[pos - 1]:
            pos += 1
        bounds.append(pos)
    bounds.append(Etot)

    cores = []
    for c in range(P):
        s0, s1 = bounds[c], bounds[c + 1]
        ssc, ddc = ss[s0:s1], dd[s0:s1]
        n0 = int(ddc[0])
        # segments (runs of equal dst)
        chg = np.nonzero(np.diff(ddc))[0] + 1
        seg_starts = np.concatenate([[0], chg])
        seg_ends = np.concatenate([chg, [ddc.shape[0]]])
        blocks = []  # each: (b0, list of (estart,eend,dstval))
        cur = None
        for a, b in zip(seg_starts, seg_ends):
            dv = int(ddc[a])
            ln = b - a
            if cur is None:
                cur = [dv, []]
                used = 0
            if dv - cur[0] >= 128 or used + ln > 1024:
                blocks.append(cur)
                cur = [dv, []]
                used = 0
            cur[1].append((a, b, dv))
            used += ln
        if cur is not None and cur[1]:
            blocks.append(cur)
        cores.append((s0, s1, n0, ssc, ddc, blocks))

    NBm = max(len(cb[5]) for cb in cores)
    cfg.NB = NBm
    DN = NBm * 128  # dense rows per core
    cfg.DN = DN
    cfg.SENT2 = P * DN
    cfg.TBL2 = cfg.SENT2 + 128

    per_core = []
    batch = np.asarray(batch, np.int64)
    for c in range(P):
        s0, s1, n0, ssc, ddc, blocks = cores[c]
        srow = np.full((NBm, 1024), cfg.SENT1, np.int32)   # table1 row of src
        drow = np.zeros((NBm, 1024), np.int32)             # table1 row of dst (for adt1 gather)
        drel = np.zeros((NBm, 1024), np.float32)           # dst - b0
        srow2 = np.full((NBm, 1024), cfg.SENT2, np.int32)  # table2 row of src
        drow2 = np.zeros((NBm, 1024), np.int32)            # LOCAL dense row of dst (adt2 local)
        nd_pos = np.full(N, -1, np.int64)                  # node -> local dense row (this core's dst only)
        for bi, (b0, segs) in enumerate(blocks):
            o = 0
            for (a, b, dv) in segs:
                ln = b - a
                srow[bi, o:o + ln] = [cfg.trow(int(v)) for v in ssc[a:b]]
                drow[bi, o:o + ln] = cfg.trow(dv)
                drel[bi, o:o + ln] = dv - b0
                nd_pos[dv] = bi * 128 + (dv - b0)
                drow2[bi, o:o + ln] = bi * 128 + (dv - b0)
                o += ln
        per_core.append(dict(n0=n0, blocks=blocks, srow=srow, drow=drow, drel=drel,
                             srow2=srow2, drow2=drow2, nd_pos=nd_pos))

    # second pass: srow2 needs the OWNING core's dense position of each src node
    nd_pos_all = np.full(N, -1, np.int64)
    node_core = np.zeros(N, np.int64)
    for c in range(P):
        m = per_core[c]["nd_pos"] >= 0
        nd_pos_all[m] = c * DN + per_core[c]["nd_pos"][m]
        node_core[m] = c
    assert (nd_pos_all >= 0).all()
    for c in range(P):
        s0, s1, n0, ssc, ddc, blocks = cores[c]
        pc = per_core[c]
        for bi, (b0, segs) in enumerate(blocks):
            o = 0
            for (a, b, dv) in segs:
                ln = b - a
                pc["srow2"][bi, o:o + ln] = nd_pos_all[ssc[a:b]]
                o += ln
        # batch per dense row (sentinel 999 where no node)
        bd = np.full(DN, 999.0, np.float32)
        m = pc["nd_pos"] >= 0
        nodes = np.nonzero(m)[0]
        bd[pc["nd_pos"][nodes]] = batch[nodes]
        pc["batch_dense"] = bd
    return per_core


def lane_layout(arr):
    """[NB,1024] -> [128, NB*8]: col b*8+t holds lanes of tile t of block b."""
    NB = arr.shape[0]
    return np.ascontiguousarray(
        arr.reshape(NB, 8, 128).transpose(2, 0, 1).reshape(128, NB * 8))


def build_inputs(cfg, x, weights, per_core):
    """Per-core input dicts for run_bass_kernel_spmd."""
    (gru_w_ih, gru_w_hh, gru_b_ih, gru_b_hh, W1, att_src1, att_dst1, b1,
     W2, att_src2, att_dst2, b2, fc_w, fc_b) = weights
    P, NPC, NPAD = cfg.P, cfg.NPC, cfg.NPC_PAD
    com = dict(
        wihT=np.ascontiguousarray(gru_w_ih.T.astype(np.float32)),           # [128,192]
        whhT=np.ascontiguousarray(gru_w_hh.T.astype(np.float32)),           # [64,192]
        bih=gru_b_ih.reshape(1, 192).astype(np.float32),
        bhh=gru_b_hh.reshape(1, 192).astype(np.float32),
        bihn=np.ascontiguousarray(gru_b_ih[128:192].reshape(64, 1).astype(np.float32)),
        W1=W1.astype(np.float32),                                           # [64,256]
        W1T=np.ascontiguousarray(W1.T.astype(np.float32)),                  # [256,64]
        as1T=np.ascontiguousarray(att_src1.T.astype(np.float32)),           # [64,4]
        ad1T=np.ascontiguousarray(att_dst1.T.astype(np.float32)),           # [64,4]
        b1b=np.broadcast_to(b1.astype(np.float32), (128, 256)).copy(),
        W2=W2.astype(np.float32),                                           # [256,64]
        W2T=np.ascontiguousarray(W2.T.astype(np.float32)),                  # [64,256]
        as2T=np.ascontiguousarray(att_src2.T.astype(np.float32)),           # [64,1]
        ad2T=np.ascontiguousarray(att_dst2.T.astype(np.float32)),           # [64,1]
        b2b=np.broadcast_to(b2.astype(np.float32), (128, 64)).copy(),
        fcw=fc_w.astype(np.float32),                                        # [64,10]
        fcbb=np.broadcast_to(fc_b.astype(np.float32), (128, 10)).copy(),
    )
    in_maps = []
    for c in range(P):
        pc = per_core[c]
        xp = np.zeros((NPAD, cfg.T, cfg.D), np.float32)
        xp[:NPC] = x[c * NPC:(c + 1) * NPC]
        bd = pc["batch_dense"]
        nch = cfg.DN // 128
        m = dict(com)
        m.update(
            xp=xp,
            srow=lane_layout(pc["srow"]),
            drow=lane_layout(pc["drow"]),
            drel=lane_layout(pc["drel"]),
            srow2=lane_layout(pc["srow2"]),
            drow2=lane_layout(pc["drow2"]),
            batch=np.ascontiguousarray(bd.reshape(nch, 128).T.copy()),      # [128, nch]
        )
        in_maps.append(m)
    return in_maps


def build_kernel(cfg, dbg=False):
    P, T, NPAD, NB, DN = cfg.P, cfg.T, cfg.NPC_PAD, cfg.NB, cfg.DN
    GT = cfg.gru_tile
    NT = NPAD // GT          # gru tiles
    NCH = DN // 128          # dense chunks
    rg = [list(range(P))]

    nc = bacc.Bacc("TRN2", target_bir_lowering=False, debug=False)
    # inputs
    xp = nc.dram_tensor("xp", [NPAD, T, 128], F32, kind="ExternalInput")
    wihT = nc.dram_tensor("wihT", [128, 192], F32, kind="ExternalInput")
    whhT = nc.dram_tensor("whhT", [64, 192], F32, kind="ExternalInput")
    bih = nc.dram_tensor("bih", [1, 192], F32, kind="ExternalInput")
    bhh = nc.dram_tensor("bhh", [1, 192], F32, kind="ExternalInput")
    bihn = nc.dram_tensor("bihn", [64, 1], F32, kind="ExternalInput")
    W1 = nc.dram_tensor("W1", [64, 256], F32, kind="ExternalInput")
    W1T = nc.dram_tensor("W1T", [256, 64], F32, kind="ExternalInput")
    as1T = nc.dram_tensor("as1T", [64, 4], F32, kind="ExternalInput")
    ad1T = nc.dram_tensor("ad1T", [64, 4], F32, kind="ExternalInput")
    b1b = nc.dram_tensor("b1b", [128, 256], F32, kind="ExternalInput")
    W2 = nc.dram_tensor("W2", [256, 64], F32, kind="ExternalInput")
    W2T = nc.dram_tensor("W2T", [64, 256], F32, kind="ExternalInput")
    as2T = nc.dram_tensor("as2T", [64, 1], F32, kind="ExternalInput")
    ad2T = nc.dram_tensor("ad2T", [64, 1], F32, kind="ExternalInput")
    b2b = nc.dram_tensor("b2b", [128, 64], F32, kind="ExternalInput")
    fcw = nc.dram_tensor("fcw", [64, 10], F32, kind="ExternalInput")
    fcbb = nc.dram_tensor("fcbb", [128, 10], F32, kind="ExternalInput")
    srow = nc.dram_tensor("srow", [128, NB * 8], I32, kind="ExternalInput")
    drow = nc.dram_tensor("drow", [128, NB * 8], I32, kind="ExternalInput")
    drel = nc.dram_tensor("drel", [128, NB * 8], F32, kind="ExternalInput")
    srow2 = nc.dram_tensor("srow2", [128, NB * 8], I32, kind="ExternalInput")
    drow2 = nc.dram_tensor("drow2", [128, NB * 8], I32, kind="ExternalInput")
    batch = nc.dram_tensor("batch", [128, NCH], F32, kind="ExternalInput")
    out = nc.dram_tensor("out", [128, 16], F32, kind="ExternalOutput")
    # internal dram
    xcat1l = nc.dram_tensor("xcat1l", [NPAD, 264], F32)
    adt1l = nc.dram_tensor("adt1l", [NPAD, 4], F32)
    shared = "Shared" if P > 4 else "Local"
    table1 = nc.dram_tensor("table1", [cfg.TBL1, 264], F32, addr_space=shared)
    adt1 = nc.dram_tensor("adt1", [cfg.TBL1, 4], F32, addr_space=shared)
    out1d = nc.dram_tensor("out1d", [DN, 260], F32)
    xcat2l = nc.dram_tensor("xcat2l", [DN, 68], F32)
    adt2l = nc.dram_tensor("adt2l", [DN, 4], F32)
    table2 = nc.dram_tensor("table2", [cfg.TBL2, 68], F32, addr_space=shared)
    out2d = nc.dram_tensor("out2d", [DN, 68], F32)
    arin = nc.dram_tensor("arin", [128, 65], F32)
    if dbg:
        dxc = nc.dram_tensor("dxc", [NPAD, 264], F32, kind="ExternalOutput")
        dtb = nc.dram_tensor("dtb", [cfg.TBL1, 264], F32, kind="ExternalOutput")
        dgt = nc.dram_tensor("dgt", [128, 16 * 264], F32, kind="ExternalOutput")
        do1 = nc.dram_tensor("do1", [DN, 260], F32, kind="ExternalOutput")
        do2 = nc.dram_tensor("do2", [DN, 68], F32, kind="ExternalOutput")
        dar = nc.dram_tensor("dar", [128, 65], F32, kind="ExternalOutput")
    arout = nc.dram_tensor("arout", [128, 65], F32, addr_space=shared)

    with tile.TileContext(nc) as tc:
        with (
            tc.tile_pool(name="pers", bufs=1) as pers,
        ):
            # ---- setup: load weights & consts ----
            ident = pers.tile([128, 128], F32)
            make_identity(nc, ident[:])
            iota_row = pers.tile([128, 128], F32)
            nc.gpsimd.iota(iota_row[:], pattern=[[1, 128]], base=0, channel_multiplier=0,
                           allow_small_or_imprecise_dtypes=True)
            wihT_s = pers.tile([128, 192], F32)
            nc.sync.dma_start(out=wihT_s[:], in_=wihT[:])
            whrz = pers.tile([65, 128], F32)
            whn = pers.tile([65, 64], F32)
            nc.sync.dma_start(out=whrz[0:64, :], in_=whhT[:, 0:128])
            nc.sync.dma_start(out=whn[0:64, :], in_=whhT[:, 128:192])
            bih_s = pers.tile([1, 192], F32)
            bhh_s = pers.tile([1, 192], F32)
            nc.sync.dma_start(out=bih_s[:], in_=bih[:])
            nc.sync.dma_start(out=bhh_s[:], in_=bhh[:])
            bsum = pers.tile([1, 192], F32)
            nc.vector.tensor_add(out=bsum[:], in0=bih_s[:], in1=bhh_s[:])
            nc.sync.dma_start(out=whrz[64:65, :], in_=bsum[0:1, 0:128])
            nc.sync.dma_start(out=whn[64:65, :], in_=bhh_s[0:1, 128:192])
            bihn_s = pers.tile([64, 1], F32)
            nc.sync.dma_start(out=bihn_s[:], in_=bihn[:])
            W1_s = pers.tile([64, 256], F32)
            nc.sync.dma_start(out=W1_s[:], in_=W1[:])
            W1T_s = pers.tile([128, 128], F32)  # reused per slice via dma below
            pp_ctx = tc.tile_pool(name="pset", bufs=1, space="PSUM")
            pp = pp_ctx.__enter__()
            # BC1 [64, 8]: cols 0:4 = B1 (src proj), 4:8 = C1 (dst proj)
            as1_s = pers.tile([64, 4], F32)
            ad1_s = pers.tile([64, 4], F32)
            nc.sync.dma_start(out=as1_s[:], in_=as1T[:])
            nc.sync.dma_start(out=ad1_s[:], in_=ad1T[:])
            BC1 = pers.tile([64, 8], F32)
            for h in range(4):
                w1blk = pers.tile([64, 64], F32, tag="w1blk", bufs=2)
                nc.sync.dma_start(out=w1blk[:], in_=W1T[64 * h:64 * (h + 1), :])
                pb = pp.tile([64, 2], F32, tag="pbc")
                nc.tensor.matmul(out=pb[:, 0:1], lhsT=w1blk[:], rhs=as1_s[:, h:h + 1],
                                 start=True, stop=True)
                nc.tensor.matmul(out=pb[:, 1:2], lhsT=w1blk[:], rhs=ad1_s[:, h:h + 1],
                                 start=True, stop=True)
                nc.vector.tensor_copy(out=BC1[:, h:h + 1], in_=pb[:, 0:1])
                nc.vector.tensor_copy(out=BC1[:, 4 + h:5 + h], in_=pb[:, 1:2])
            b1b_s = pers.tile([128, 256], F32)
            nc.sync.dma_start(out=b1b_s[:], in_=b1b[:])
            # [256,64] stored as [128, 128]: cols 0:64 = W2[0:128], 64:128 = W2[128:256]
            W2_s = pers.tile([128, 128], F32, name="W2s")
            nc.sync.dma_start(out=W2_s[:, 0:64], in_=W2[0:128, :])
            nc.sync.dma_start(out=W2_s[:, 64:128], in_=W2[128:256, :])
            # BC2 [128, 4]: cols 0:2 = [B2|C2][0:128], cols 2:4 = [B2|C2][128:256]
            as2_s = pers.tile([64, 1], F32)
            ad2_s = pers.tile([64, 1], F32)
            nc.sync.dma_start(out=as2_s[:], in_=as2T[:])
            nc.sync.dma_start(out=ad2_s[:], in_=ad2T[:])
            BC2 = pers.tile([128, 4], F32)
            for half in range(2):
                w2blk = pers.tile([64, 128], F32, tag="w2blk", bufs=2)
                nc.sync.dma_start(out=w2blk[:], in_=W2T[:, 128 * half:128 * (half + 1)])
                pb2 = pp.tile([128, 2], F32, tag="pbc2")
                nc.tensor.matmul(out=pb2[:, 0:1], lhsT=w2blk[:], rhs=as2_s[:],
                                 start=True, stop=True)
                nc.tensor.matmul(out=pb2[:, 1:2], lhsT=w2blk[:], rhs=ad2_s[:],
                                 start=True, stop=True)
                nc.vector.tensor_copy(out=BC2[:, 2 * half:2 * half + 2], in_=pb2[:])
            b2b_s = pers.tile([128, 64], F32)
            nc.sync.dma_start(out=b2b_s[:], in_=b2b[:])
            fcw_s = pers.tile([64, 10], F32)
            nc.sync.dma_start(out=fcw_s[:], in_=fcw[:])
            fcbb_s = pers.tile([128, 10], F32)
            nc.sync.dma_start(out=fcbb_s[:], in_=fcbb[:])
            batch_s = pers.tile([128, NCH], F32)
            nc.sync.dma_start(out=batch_s[:], in_=batch[:])
            pp_ctx.__exit__(None, None, None)

            # ---- phase 1: GRU + xcat1/adt1 ----
            with (
                tc.tile_pool(name="g1", bufs=2) as g1,
                tc.tile_pool(name="g2", bufs=3) as g2,
                tc.tile_pool(name="g3", bufs=2) as g3,
                tc.tile_pool(name="gps", bufs=2, space="PSUM") as gps,
                tc.tile_pool(name="gps2", bufs=1, space="PSUM") as gps2,
            ):
                NCG = GT // 128
                for it in range(NT):
                    xt = g1.tile([128, NCG, T * 128], F32, tag="xt")
                    for cc in range(NCG):
                        nc.sync.dma_start(
                            out=xt[:, cc, :],
                            in_=xp[it * GT + cc * 128: it * GT + (cc + 1) * 128, :, :])
                    xT = g2.tile([128, T, GT], F32, tag="xT")
                    for t in range(T):
                        for cc in range(NCG):
                            pt = gps2.tile([128, 128], F32, tag="pt", bufs=2)
                            nc.tensor.transpose(out=pt[:], in_=xt[:, cc, t * 128:(t + 1) * 128],
                                                identity=ident[:])
                            nc.scalar.copy(out=xT[:, t, cc * 128:(cc + 1) * 128], in_=pt[:])
                    hT = g2.tile([65, GT], F32, tag="hT")
                    nc.vector.memset(hT[0:64, :], 0.0)
                    nc.vector.memset(hT[64:65, :], 1.0)
                    for t in range(T):
                        prz = gps.tile([64, 2 * GT], F32, tag="prz", bufs=1)
                        nc.tensor.matmul(out=prz[:, 0:GT], lhsT=wihT_s[:, 0:64],
                                         rhs=xT[:, t, :], start=True, stop=False)
                        nc.tensor.matmul(out=prz[:, 0:GT], lhsT=whrz[:, 0:64], rhs=hT[:],
                                         start=False, stop=True)
                        nc.tensor.matmul(out=prz[:, GT:2 * GT], lhsT=wihT_s[:, 64:128],
                                         rhs=xT[:, t, :], start=True, stop=False)
                        nc.tensor.matmul(out=prz[:, GT:2 * GT], lhsT=whrz[:, 64:128],
                                         rhs=hT[:], start=False, stop=True)
                        pin = gps2.tile([64, GT], F32, tag="pin")
                        nc.tensor.matmul(out=pin[:], lhsT=wihT_s[:, 128:192], rhs=xT[:, t, :],
                                         start=True, stop=True)
                        phn = gps2.tile([64, GT], F32, tag="phn")
                        nc.tensor.matmul(out=phn[:], lhsT=whn[:], rhs=hT[:],
                                         start=True, stop=True)
                        rz = g3.tile([64, 2 * GT], F32, tag="rz")
                        nc.scalar.activation(out=rz[:], in_=prz[:], func=AF.Sigmoid)
                        tmp = g3.tile([64, GT], F32, tag="tmp")
                        nc.vector.tensor_mul(out=tmp[:], in0=rz[:, 0:GT], in1=phn[:])
                        t3 = g3.tile([64, GT], F32, tag="t3")
                        nc.vector.tensor_add(out=t3[:], in0=pin[:], in1=tmp[:])
                        nh = g3.tile([64, GT], F32, tag="nh")
                        nc.scalar.activation(out=nh[:], in_=t3[:], func=AF.Tanh,
                                             bias=bihn_s[:, 0:1])
                        s1 = g3.tile([64, GT], F32, tag="s1")
                        nc.vector.tensor_sub(out=s1[:], in0=hT[0:64, :], in1=nh[:])
                        s2 = g3.tile([64, GT], F32, tag="s2")
                        nc.vector.tensor_mul(out=s2[:], in0=rz[:, GT:2 * GT], in1=s1[:])
                        nc.vector.tensor_add(out=hT[0:64, :], in0=nh[:], in1=s2[:])
                    for cc in range(NCG):
                        pxs = gps.tile([128, 256], F32, tag="pxs", bufs=1)
                        nc.tensor.matmul(out=pxs[:], lhsT=hT[0:64, cc * 128:(cc + 1) * 128],
                                         rhs=W1_s[:], start=True, stop=True)
                        pat = gps2.tile([128, 8], F32, tag="pat")
                        nc.tensor.matmul(out=pat[:], lhsT=hT[0:64, cc * 128:(cc + 1) * 128],
                                         rhs=BC1[:], start=True, stop=True)
                        xc = g3.tile([128, 264], F32, tag="xc")
                        nc.scalar.copy(out=xc[:, 0:256], in_=pxs[:])
                        nc.vector.tensor_copy(out=xc[:, 256:260], in_=pat[:, 0:4])
                        nc.vector.memset(xc[:, 260:264], 0.0)
                        ad4 = g3.tile([128, 4], F32, tag="ad4")
                        nc.vector.tensor_copy(out=ad4[:], in_=pat[:, 4:8])
                        r0 = it * GT + cc * 128
                        nc.sync.dma_start(out=xcat1l[r0:r0 + 128, :], in_=xc[:])
                        nc.sync.dma_start(out=adt1l[r0:r0 + 128, :], in_=ad4[:])

            # ---- AllGather xcat1/adt1, sentinel ----
            tc.strict_bb_all_engine_barrier()
            nc.gpsimd.collective_compute(
                "AllGather", OP.bypass, replica_groups=rg,
                ins=[xcat1l[:]], outs=[table1[0:P * NPAD, :]])
            nc.gpsimd.collective_compute(
                "AllGather", OP.bypass, replica_groups=rg,
                ins=[adt1l[:]], outs=[adt1[0:P * NPAD, :]])
            sent = pers.tile([1, 264], F32)
            nc.vector.memset(sent[:], 0.0)
            nc.vector.memset(sent[:, 256:260], -100.0)
            nc.sync.dma_start(out=table1[cfg.SENT1:cfg.SENT1 + 1, :], in_=sent[:])
            tc.strict_bb_all_engine_barrier()

            if dbg:
                nc.sync.dma_start(out=dxc[:], in_=xcat1l[:])
                nc.sync.dma_start(out=dtb[:], in_=table1[:])
            # ---- phase 2: GAT1 over edge blocks ----
            def gat_layer(tblR, adtR, idxs, idxd, ncols, nheads, outd, payw, acol, dump=None):
                """ncols: gathered row width; payw: payload width (= nheads + nheads*64
                padded start at col 4)."""
                with (
                    tc.tile_pool(name="eg", bufs=2) as eg,
                    tc.tile_pool(name="em", bufs=6) as em,
                    tc.tile_pool(name="es", bufs=6) as es,
                    tc.tile_pool(name="eps", bufs=2, space="PSUM") as eps,
                ):
                    idx_s = pers.tile([128, NB * 8], I32, tag=f"idxs{ncols}", name=f"idxs{ncols}")
                    nc.sync.dma_start(out=idx_s[:], in_=idxs[:])
                    idxd_s = pers.tile([128, NB * 8], I32, tag=f"idxd{ncols}", name=f"idxd{ncols}")
                    nc.sync.dma_start(out=idxd_s[:], in_=idxd[:])
                    drel_s = pers.tile([128, NB * 8], F32, tag=f"drel{ncols}", name=f"drel{ncols}")
                    nc.sync.dma_start(out=drel_s[:], in_=drel[:])
                    NBG = 2  # blocks per gather
                    for bg in range(NB // NBG + (1 if NB % NBG else 0)):
                        blo = bg * NBG
                        bn = min(NBG, NB - blo)
                        g = eg.tile([128, NBG * 8, ncols], F32, tag="g")
                        ga = eg.tile([128, NBG * 8, 4], F32, tag="ga")
                        for q in range(bn * 8):
                            nc.gpsimd.indirect_dma_start(
                                out=g[:, q, :], out_offset=None, in_=tblR[:],
                                in_offset=bass.IndirectOffsetOnAxis(
                                    ap=idx_s[:, blo * 8 + q:blo * 8 + q + 1], axis=0))
                            nc.gpsimd.indirect_dma_start(
                                out=ga[:, q, :], out_offset=None, in_=adtR[:],
                                in_offset=bass.IndirectOffsetOnAxis(
                                    ap=idxd_s[:, blo * 8 + q:blo * 8 + q + 1], axis=0))
                        if dump is not None and bg == 0:
                            nc.sync.dma_start(out=dump[:], in_=g[:].rearrange("p a b -> p (a b)"))
                        u = es.tile([128, NBG * 8, 4], F32, tag="u")
                        nc.vector.tensor_add(out=u[:, 0:bn * 8, 0:nheads],
                                             in0=g[:, 0:bn * 8, acol:acol + nheads],
                                             in1=ga[:, 0:bn * 8, 0:nheads])
                        e1 = es.tile([128, NBG * 8, 4], F32, tag="e1")
                        nc.scalar.activation(out=e1[:, 0:bn * 8, 0:nheads],
                                             in_=u[:, 0:bn * 8, 0:nheads], func=AF.Exp)
                        e2 = es.tile([128, NBG * 8, 4], F32, tag="e2")
                        nc.scalar.activation(out=e2[:, 0:bn * 8, 0:nheads],
                                             in_=u[:, 0:bn * 8, 0:nheads], func=AF.Exp,
                                             scale=0.2)
                        ee = es.tile([128, NBG * 8, 4], F32, tag="ee")
                        nc.vector.tensor_tensor(out=ee[:, 0:bn * 8, 0:nheads],
                                                in0=e1[:, 0:bn * 8, 0:nheads],
                                                in1=e2[:, 0:bn * 8, 0:nheads],
                                                op=OP.max)
                        for bs in range(bn):
                            b = blo + bs
                            pblk = eps.tile([128, payw], F32, tag="pblk")
                            for t in range(8):
                                tt = bs * 8 + t
                                col = b * 8 + t
                                S = em.tile([128, 128], F32, tag="S")
                                nc.vector.tensor_scalar(
                                    out=S[:], in0=iota_row[:],
                                    scalar1=drel_s[:, col:col + 1], scalar2=None,
                                    op0=OP.is_equal)
                                m = em.tile([128, payw], F32, tag="m")
                                nc.vector.tensor_copy(out=m[:, 0:nheads],
                                                      in_=ee[:, tt, 0:nheads])
                                if nheads < 4:
                                    nc.vector.memset(m[:, nheads:4], 0.0)
                                for h in range(nheads):
                                    nc.vector.tensor_scalar(
                                        out=m[:, 4 + 64 * h:4 + 64 * (h + 1)],
                                        in0=g[:, tt, 64 * h:64 * (h + 1)],
                                        scalar1=ee[:, tt, h:h + 1], scalar2=None,
                                        op0=OP.mult)
                                nc.tensor.matmul(out=pblk[:], lhsT=S[:], rhs=m[:],
                                                 start=(t == 0), stop=(t == 7))
                            ob = es.tile([128, payw], F32, tag="ob")
                            nc.scalar.copy(out=ob[:], in_=pblk[:])
                            nc.sync.dma_start(out=outd[b * 128:(b + 1) * 128, :], in_=ob[:])

            gat_layer(table1, adt1, srow, drow, 264, 4, out1d, 260, 256, dump=(dgt if dbg else None))

            tc.strict_bb_all_engine_barrier()
            if dbg:
                nc.sync.dma_start(out=do1[:], in_=out1d[:])
            # ---- phase 3: h1, xcat2, adt2 ----
            with (
                tc.tile_pool(name="h3", bufs=4) as h3,
                tc.tile_pool(name="h3p", bufs=2, space="PSUM") as h3p,
                tc.tile_pool(name="h3q", bufs=2, space="PSUM") as h3q,
            ):
                for ch in range(NCH):
                    o1 = h3.tile([128, 260], F32, tag="o1")
                    nc.sync.dma_start(out=o1[:], in_=out1d[ch * 128:(ch + 1) * 128, :])
                    dmx = h3.tile([128, 4], F32, tag="dmx")
                    nc.vector.tensor_scalar(out=dmx[:], in0=o1[:, 0:4], scalar1=1e-12,
                                            scalar2=None, op0=OP.max)
                    rec = h3.tile([128, 4], F32, tag="rec")
                    nc.vector.reciprocal(out=rec[:], in_=dmx[:])
                    h1 = h3.tile([128, 256], F32, tag="h1")
                    for h in range(4):
                        nc.vector.tensor_scalar(
                            out=h1[:, 64 * h:64 * (h + 1)],
                            in0=o1[:, 4 + 64 * h:4 + 64 * (h + 1)],
                            scalar1=rec[:, h:h + 1], scalar2=None, op0=OP.mult)
                    nc.vector.tensor_add(out=h1[:], in0=h1[:], in1=b1b_s[:])
                    nc.vector.tensor_scalar(out=h1[:], in0=h1[:], scalar1=0.0,
                                            scalar2=None, op0=OP.max)
                    tp = h3.tile([128, 256], F32, tag="tp")  # h1T halves side by side
                    for half in range(2):
                        ptp = h3q.tile([128, 128], F32, tag="ptp")
                        nc.tensor.transpose(out=ptp[:], in_=h1[:, 128 * half:128 * (half + 1)],
                                            identity=ident[:])
                        nc.scalar.copy(out=tp[:, 128 * half:128 * (half + 1)], in_=ptp[:])
                    pxs2 = h3p.tile([128, 64], F32, tag="pxs2")
                    pat2 = h3q.tile([128, 2], F32, tag="pat2")
                    for half in range(2):
                        nc.tensor.matmul(out=pxs2[:], lhsT=tp[:, 128 * half:128 * (half + 1)],
                                         rhs=W2_s[:, 64 * half:64 * (half + 1)],
                                         start=(half == 0), stop=(half == 1))
                        nc.tensor.matmul(out=pat2[:], lhsT=tp[:, 128 * half:128 * (half + 1)],
                                         rhs=BC2[:, 2 * half:2 * half + 2],
                                         start=(half == 0), stop=(half == 1))
                    xc2 = h3.tile([128, 68], F32, tag="xc2")
                    nc.scalar.copy(out=xc2[:, 0:64], in_=pxs2[:])
                    nc.vector.tensor_copy(out=xc2[:, 64:65], in_=pat2[:, 0:1])
                    nc.vector.memset(xc2[:, 65:68], 0.0)
                    ad42 = h3.tile([128, 4], F32, tag="ad42")
                    nc.vector.tensor_copy(out=ad42[:, 0:1], in_=pat2[:, 1:2])
                    nc.vector.memset(ad42[:, 1:4], 0.0)
                    nc.sync.dma_start(out=xcat2l[ch * 128:(ch + 1) * 128, :], in_=xc2[:])
                    nc.sync.dma_start(out=adt2l[ch * 128:(ch + 1) * 128, :], in_=ad42[:])

            tc.strict_bb_all_engine_barrier()
            nc.gpsimd.collective_compute(
                "AllGather", OP.bypass, replica_groups=rg,
                ins=[xcat2l[:]], outs=[table2[0:P * DN, :]])
            sent2 = pers.tile([1, 68], F32)
            nc.vector.memset(sent2[:], 0.0)
            nc.vector.memset(sent2[:, 64:65], -100.0)
            nc.sync.dma_start(out=table2[cfg.SENT2:cfg.SENT2 + 1, :], in_=sent2[:])
            tc.strict_bb_all_engine_barrier()

            # ---- phase 4: GAT2 ----
            gat_layer(table2, adt2l, srow2, drow2, 68, 1, out2d, 68, 64)

            tc.strict_bb_all_engine_barrier()
            if dbg:
                nc.sync.dma_start(out=do2[:], in_=out2d[:])
            # ---- phase 5: h2, pooling, fc, log_softmax ----
            with (
                tc.tile_pool(name="r5", bufs=4) as r5,
                tc.tile_pool(name="r5p", bufs=1, space="PSUM") as r5p,
                tc.tile_pool(name="r5q", bufs=2, space="PSUM") as r5q,
            ):
                ppool = r5p.tile([128, 65], F32)
                for ch in range(NCH):
                    o2 = r5.tile([128, 68], F32, tag="o2")
                    nc.sync.dma_start(out=o2[:], in_=out2d[ch * 128:(ch + 1) * 128, :])
                    dm2 = r5.tile([128, 1], F32, tag="dm2")
                    nc.vector.tensor_scalar(out=dm2[:], in0=o2[:, 0:1], scalar1=1e-12,
                                            scalar2=None, op0=OP.max)
                    rc2 = r5.tile([128, 1], F32, tag="rc2")
                    nc.vector.reciprocal(out=rc2[:], in_=dm2[:])
                    ph = r5.tile([128, 68], F32, tag="ph")
                    nc.vector.tensor_scalar(out=ph[:, 0:64], in0=o2[:, 4:68],
                                            scalar1=rc2[:, 0:1], scalar2=None, op0=OP.mult)
                    nc.vector.tensor_add(out=ph[:, 0:64], in0=ph[:, 0:64], in1=b2b_s[:])
                    nc.vector.tensor_scalar(out=ph[:, 0:64], in0=ph[:, 0:64], scalar1=0.0,
                                            scalar2=None, op0=OP.max)
                    nc.vector.memset(ph[:, 64:65], 1.0)
                    Sb = r5.tile([128, 128], F32, tag="Sb")
                    nc.vector.tensor_scalar(out=Sb[:], in0=iota_row[:],
                                            scalar1=batch_s[:, ch:ch + 1], scalar2=None,
                                            op0=OP.is_equal)
                    nc.tensor.matmul(out=ppool[:], lhsT=Sb[:], rhs=ph[:, 0:65],
                                     start=(ch == 0), stop=(ch == NCH - 1))
                pr = r5.tile([128, 65], F32)
                nc.scalar.copy(out=pr[:], in_=ppool[:])
                nc.sync.dma_start(out=arin[:], in_=pr[:])
                tc.strict_bb_all_engine_barrier()
                nc.gpsimd.collective_compute(
                    "AllReduce", OP.add, replica_groups=rg,
                    ins=[arin[:]], outs=[arout[:]])
                tc.strict_bb_all_engine_barrier()
                ar = r5.tile([128, 65], F32)
                nc.sync.dma_start(out=ar[:], in_=arout[:])
                if dbg:
                    nc.sync.dma_start(out=dar[:], in_=ar[:])
                cm = r5.tile([128, 1], F32)
                nc.vector.tensor_scalar(out=cm[:], in0=ar[:, 64:65], scalar1=1.0,
                                        scalar2=None, op0=OP.max)
                cr = r5.tile([128, 1], F32)
                nc.vector.reciprocal(out=cr[:], in_=cm[:])
                gf = r5.tile([128, 64], F32)
                nc.vector.tensor_scalar(out=gf[:], in0=ar[:, 0:64], scalar1=cr[:, 0:1],
                                        scalar2=None, op0=OP.mult)
                pgt = r5q.tile([64, 128], F32)
                nc.tensor.transpose(out=pgt[:], in_=gf[:], identity=ident[:])
                gfT = r5.tile([64, 128], F32)
                nc.scalar.copy(out=gfT[:], in_=pgt[:])
                plg = r5q.tile([128, 10], F32)
                nc.tensor.matmul(out=plg[:], lhsT=gfT[:], rhs=fcw_s[:], start=True, stop=True)
                lg = r5.tile([128, 16], F32)
                nc.vector.tensor_add(out=lg[:, 0:10], in0=plg[:], in1=fcbb_s[:])
                mx = r5.tile([128, 1], F32)
                nc.vector.reduce_max(out=mx[:], in_=lg[:, 0:10], axis=mybir.AxisListType.X)
                tsh = r5.tile([128, 16], F32)
                nc.vector.tensor_scalar(out=tsh[:, 0:10], in0=lg[:, 0:10],
                                        scalar1=mx[:, 0:1], scalar2=None, op0=OP.subtract)
                exs = r5.tile([128, 16], F32)
                se = r5.tile([128, 1], F32)
                nc.scalar.activation(out=exs[:, 0:10], in_=tsh[:, 0:10], func=AF.Exp,
                                     accum_out=se[:])
                ln = r5.tile([128, 1], F32)
                nc.scalar.activation(out=ln[:], in_=se[:], func=AF.Ln)
                res = r5.tile([128, 16], F32)
                nc.vector.memset(res[:], 0.0)
                nc.vector.tensor_scalar(out=res[:, 0:10], in0=tsh[:, 0:10],
                                        scalar1=ln[:, 0:1], scalar2=None, op0=OP.subtract)
                nc.sync.dma_start(out=out[:], in_=res[:])
    nc.compile()
    return nc


def run_full(x, edge_index, batch, weights, cfg=None, core_ids=None):
    from concourse.bass_utils import run_bass_kernel_spmd
    if cfg is None:
        cfg = Cfg(100000, 1600000, 128, 8, gru_tile=512)
    per_core = host_prep(cfg, edge_index, batch)
    in_maps = build_inputs(cfg, x, weights, per_core)
    nc = build_kernel(cfg)
    if core_ids is None:
        core_ids = list(range(cfg.P))
    res = run_bass_kernel_spmd(nc, in_maps, core_ids=core_ids)
    return res.results[0]["out"][:cfg.G, :10], res


# ---------------- self-contained entry point ----------------
_CACHE = {}


def kernel(**inputs):
    """Full DAGNN forward. Takes the unsharded inputs from setup_inputs();
    returns log-softmax output [num_graphs, 10] float32."""
    x = np.asarray(inputs["x"], np.float32)
    edge_index = np.asarray(inputs["edge_index"])
    batch = np.asarray(inputs["batch"])
    G = int(inputs["num_graphs"])
    weights = [np.asarray(inputs[k], np.float32) for k in (
        "gru_w_ih", "gru_w_hh", "gru_b_ih", "gru_b_hh",
        "W1", "att_src1", "att_dst1", "b1",
        "W2", "att_src2", "att_dst2", "b2", "fc_w", "fc_b")]
    N = x.shape[0]
    E = edge_index.shape[1]
    P = 8

    from concourse.bass_utils import run_bass_kernel_spmd
    cfg = Cfg(N, E, G, P, gru_tile=512)
    per_core = host_prep(cfg, edge_index, batch)
    in_maps = build_inputs(cfg, x, weights, per_core)
    key = (N, E, G, P, cfg.NB)
    if key not in _CACHE:
        _CACHE[key] = build_kernel(cfg)
    nc = _CACHE[key]
    res = run_bass_kernel_spmd(nc, in_maps, core_ids=list(range(P)))
    out = np.asarray(res.results[0]["out"][:G, :10], np.float32)
    return out



# revision 10
# speedup vs baseline: 2.5801x; 2.5801x over previous
"""DAGNN (GRU + 2xGAT + mean-pool + fc + log_softmax) on 8 TRN2 cores via Bass/Tile.

Sharding: nodes split across cores by dst-range (edges sorted by dst, split at
dst boundaries), so each core's GRU computes exactly the h/attention values its
GAT dst windows need locally. Edge payload gathers use batched dma_gather from
a 4-way row-sharded bf16 table (int16 index limit); per-window dst attention
terms are expanded on-chip via selection-matrix matmuls. Feature tables are
AllGathered; graph pooling partial sums are AllReduced.
"""
import sys
import numpy as np

sys.path.insert(0, "/opt/trn_rl_repo")

import ml_dtypes
import concourse.bass as bass
import concourse.bacc as bacc
import concourse.mybir as mybir
import concourse.tile as tile
from concourse.masks import make_identity

F32 = mybir.dt.float32
BF16 = mybir.dt.bfloat16
I16 = mybir.dt.int16
U8 = mybir.dt.uint8
AF = mybir.ActivationFunctionType
OP = mybir.AluOpType

NSH = 4          # table row shards (int16 gather index limit)
GT = 512         # GRU node tile


def _ceil(a, b):
    return -(-a // b)


class Cfg:
    def __init__(self, N, E, G, P):
        self.N, self.E, self.G, self.P = N, E, G, P
        self.T, self.D, self.H = 8, 128, 64
        self.HEADS, self.C1, self.C2 = 4, 256, 64


def host_prep(cfg, edge_index, batch):
    N, E, P = cfg.N, cfg.E, cfg.P
    src = np.concatenate([np.asarray(edge_index[0], np.int64), np.arange(N, dtype=np.int64)])
    dst = np.concatenate([np.asarray(edge_index[1], np.int64), np.arange(N, dtype=np.int64)])
    order = np.argsort(dst, kind="stable")
    ss, dd = src[order], dst[order]
    Etot = ss.shape[0]

    bounds = [0]
    for k in range(1, P):
        pos = (k * Etot) // P
        while pos < Etot and dd[pos] == dd[pos - 1]:
            pos += 1
        bounds.append(pos)
    bounds.append(Etot)
    n0 = np.zeros(P + 1, np.int64)
    n0[P] = N
    for c in range(1, P):
        n0[c] = dd[bounds[c]]
    ranges = np.diff(n0)
    NPAD2 = _ceil(int(ranges.max()), GT) * GT
    NW = NPAD2 // 128
    SH = (P * NPAD2) // NSH
    assert SH - 1 <= 32767, f"shard too large for int16: {SH}"
    cfg.n0, cfg.NPAD2, cfg.NW, cfg.SH = n0, NPAD2, NW, SH
    cfg.NT = NPAD2 // GT

    owner = np.searchsorted(n0[1:P], np.arange(N), side="right")
    g2r = owner * NPAD2 + (np.arange(N) - n0[owner])
    shard_of = (g2r // SH).astype(np.int64)
    rel_of = (g2r % SH).astype(np.int16)

    # pass 1: per-(window, shard) edge counts per core -> uniform tile counts
    NB = NW * NSH
    kws = np.zeros((P, NB), np.int64)
    per_edges = []
    for c in range(P):
        sl = slice(bounds[c], bounds[c + 1])
        ssc, ddc = ss[sl], dd[sl]
        w_arr = (ddc - n0[c]) // 128
        s_arr = shard_of[ssc]
        key = (w_arr * NSH + s_arr).astype(np.int64)
        kws[c] = np.bincount(key, minlength=NB)
        per_edges.append((ssc, ddc, w_arr, key))
    tiles = np.maximum(1, _ceil(kws.max(axis=0), 128)).astype(np.int64)
    tile_off = np.concatenate([[0], np.cumsum(tiles)])
    TOT_TILES = int(tile_off[-1])
    cfg.tiles, cfg.tile_off, cfg.TOT_TILES = tiles, tile_off, TOT_TILES
    cfg.TBMAX = int(tiles.max())

    per_core = []
    for c in range(P):
        ssc, ddc, w_arr, key = per_edges[c]
        order2 = np.argsort(key, kind="stable")
        sk = key[order2]
        grp_start = np.searchsorted(sk, np.arange(NB))
        rank = np.arange(sk.shape[0]) - grp_start[sk]
        slotpos = tile_off[sk] * 128 + rank
        TOT_SLOT = TOT_TILES * 128
        srel = np.zeros(TOT_SLOT, np.int16)
        drel = np.full(TOT_SLOT, 255, np.uint8)
        srel[slotpos] = rel_of[ssc[order2]]
        drel[slotpos] = (ddc[order2] - n0[c] - 128 * w_arr[order2]).astype(np.uint8)
        # wrapped gather indices: idx i of a (tile-aligned) run at [i%16, i//16]
        wr = np.ascontiguousarray(srel.reshape(TOT_SLOT // 16, 16).T)
        idx_wr = np.tile(wr, (8, 1))                                   # [128, TOT_SLOT//16]
        drel_pt = np.ascontiguousarray(drel.reshape(TOT_TILES, 128).T)  # [128, TOT_TILES]
        drelT = drel.reshape(1, TOT_TILES, 128).copy()                  # [1, TOT_TILES, 128]
        bd = np.full(NPAD2, 999.0, np.float32)
        rg = int(ranges[c])
        bd[:rg] = np.asarray(batch, np.int64)[n0[c]:n0[c + 1]].astype(np.float32)
        batch_wd = np.ascontiguousarray(bd.reshape(NW, 128).T)          # [128, NW]
        per_core.append(dict(idx_wr=idx_wr, drel_pt=drel_pt, drelT=drelT,
                             batch_wd=batch_wd, rg=rg))
    return per_core


def build_inputs(cfg, x, weights, per_core):
    (gru_w_ih, gru_w_hh, gru_b_ih, gru_b_hh, W1, att_src1, att_dst1, b1,
     W2, att_src2, att_dst2, b2, fc_w, fc_b) = weights
    P, NPAD2 = cfg.P, cfg.NPAD2
    bf = ml_dtypes.bfloat16

    # BC1 [64, 8]: cols 0:4 src-att coeffs per head, 4:8 dst-att
    BC1 = np.zeros((64, 8), np.float32)
    for h in range(4):
        Wh = W1[:, 64 * h:64 * (h + 1)]
        BC1[:, h] = Wh @ att_src1[h]
        BC1[:, 4 + h] = Wh @ att_dst1[h]
    # W1 block-diagonal pairs for transposed apply
    W12A = np.zeros((128, 128), np.float32)
    W12B = np.zeros((128, 128), np.float32)
    W12A[0:64, 0:64] = W1[:, 0:64]
    W12A[64:128, 64:128] = W1[:, 64:128]
    W12B[0:64, 0:64] = W1[:, 128:192]
    W12B[64:128, 64:128] = W1[:, 192:256]
    # W2 halves side by side; BC2 [128, 4]: cols 2h = [src|dst] coeffs, half h
    W2s = np.zeros((128, 128), np.float32)
    W2s[:, 0:64] = W2[0:128, :]
    W2s[:, 64:128] = W2[128:256, :]
    a2 = W2 @ att_src2[0]   # [256]
    d2 = W2 @ att_dst2[0]
    BC2 = np.zeros((128, 4), np.float32)
    BC2[:, 0] = a2[0:128]
    BC2[:, 1] = d2[0:128]
    BC2[:, 2] = a2[128:256]
    BC2[:, 3] = d2[128:256]

    com = dict(
        wihT=np.ascontiguousarray(gru_w_ih.T).astype(bf),               # [128,192]
        whrz=np.concatenate([gru_w_hh.T[:, 0:128],
                             (gru_b_ih + gru_b_hh)[None, 0:128]], 0).astype(bf),  # [65,128]
        whn=np.concatenate([gru_w_hh.T[:, 128:192],
                            gru_b_hh[None, 128:192]], 0).astype(bf),    # [65,64]
        bihn=np.ascontiguousarray(gru_b_ih[128:192].reshape(64, 1)).astype(np.float32),
        BC1=BC1.astype(bf),
        W12A=W12A.astype(bf), W12B=W12B.astype(bf),
        b1b=np.broadcast_to(b1, (128, 256)).astype(bf).copy(),
        W2s=W2s.astype(bf), BC2=BC2.astype(bf),
        b2b=np.broadcast_to(b2, (128, 64)).astype(bf).copy(),
        fcw=fc_w.astype(np.float32),
        fcbb=np.broadcast_to(fc_b, (128, 10)).astype(np.float32).copy(),
    )
    in_maps = []
    for c in range(P):
        pc = per_core[c]
        rg = pc["rg"]
        xp = np.zeros((NPAD2, cfg.T, cfg.D), np.float32)
        xp[:rg] = x[cfg.n0[c]:cfg.n0[c + 1]]
        xpT = np.ascontiguousarray(xp.transpose(1, 2, 0)).astype(bf)     # [8,128,NPAD2]
        m = dict(com)
        m.update(xpT=xpT, idx_wr=pc["idx_wr"], drel_pt=pc["drel_pt"],
                 drelT=pc["drelT"], batch_wd=pc["batch_wd"])
        in_maps.append(m)
    return in_maps


def build_kernel(cfg, dbg=False):
    P, T, NPAD2, NW, SH = cfg.P, cfg.T, cfg.NPAD2, cfg.NW, cfg.SH
    NT, NSHARD = cfg.NT, NSH
    tiles, tile_off, TOT_TILES = cfg.tiles, cfg.tile_off, cfg.TOT_TILES
    TOT_SLOT = TOT_TILES * 128
    rg_all = [list(range(P))]

    nc = bacc.Bacc("TRN2", target_bir_lowering=False, debug=False,
                   dynamic_dma_scratch_size=32768)
    # inputs
    xpT = nc.dram_tensor("xpT", [T, 128, NPAD2], BF16, kind="ExternalInput")
    wihT = nc.dram_tensor("wihT", [128, 192], BF16, kind="ExternalInput")
    whrz = nc.dram_tensor("whrz", [65, 128], BF16, kind="ExternalInput")
    whn = nc.dram_tensor("whn", [65, 64], BF16, kind="ExternalInput")
    bihn = nc.dram_tensor("bihn", [64, 1], F32, kind="ExternalInput")
    BC1 = nc.dram_tensor("BC1", [64, 8], BF16, kind="ExternalInput")
    W12A = nc.dram_tensor("W12A", [128, 128], BF16, kind="ExternalInput")
    W12B = nc.dram_tensor("W12B", [128, 128], BF16, kind="ExternalInput")
    b1b = nc.dram_tensor("b1b", [128, 256], BF16, kind="ExternalInput")
    W2s = nc.dram_tensor("W2s", [128, 128], BF16, kind="ExternalInput")
    BC2 = nc.dram_tensor("BC2", [128, 4], BF16, kind="ExternalInput")
    b2b = nc.dram_tensor("b2b", [128, 64], BF16, kind="ExternalInput")
    fcw = nc.dram_tensor("fcw", [64, 10], F32, kind="ExternalInput")
    fcbb = nc.dram_tensor("fcbb", [128, 10], F32, kind="ExternalInput")
    idx_wr = nc.dram_tensor("idx_wr", [128, TOT_SLOT // 16], I16, kind="ExternalInput")
    drel_pt = nc.dram_tensor("drel_pt", [128, TOT_TILES], U8, kind="ExternalInput")
    drelT = nc.dram_tensor("drelT", [1, TOT_TILES, 128], U8, kind="ExternalInput")
    batch_wd = nc.dram_tensor("batch_wd", [128, NW], F32, kind="ExternalInput")
    out = nc.dram_tensor("out", [128, 16], F32, kind="ExternalOutput")
    # internal dram
    xcat1l = nc.dram_tensor("xcat1l", [NPAD2, 128], BF16)
    table1 = nc.dram_tensor("table1", [P * NPAD2, 128], BF16, addr_space="Shared")
    tb1s = [nc.dram_tensor(f"tb1s{s}", [SH, 128], BF16) for s in range(NSHARD)]
    xcat2l = nc.dram_tensor("xcat2l", [NPAD2, 128], BF16)
    table2 = nc.dram_tensor("table2", [P * NPAD2, 128], BF16, addr_space="Shared")
    tb2s = [nc.dram_tensor(f"tb2s{s}", [SH, 128], BF16) for s in range(NSHARD)]
    arin = nc.dram_tensor("arin", [128, 65], F32)
    arout = nc.dram_tensor("arout", [128, 65], F32, addr_space="Shared")

    with tile.TileContext(nc) as tc:
        with tc.tile_pool(name="pers", bufs=1) as pers:
            # ---- persistent: weights, indices, iotas ----
            identb = pers.tile([128, 128], BF16)
            make_identity(nc, identb[:])
            iota_row = pers.tile([128, 128], U8)
            nc.gpsimd.iota(iota_row[:], pattern=[[1, 128]], base=0, channel_multiplier=0,
                           allow_small_or_imprecise_dtypes=True)
            iota_p = pers.tile([128, 1], F32)
            nc.gpsimd.iota(iota_p[:], pattern=[[0, 1]], base=0, channel_multiplier=1,
                           allow_small_or_imprecise_dtypes=True)
            wihT_s = pers.tile([128, 192], BF16)
            nc.sync.dma_start(out=wihT_s[:], in_=wihT[:])
            whrz_s = pers.tile([65, 128], BF16)
            nc.sync.dma_start(out=whrz_s[:], in_=whrz[:])
            whn_s = pers.tile([65, 64], BF16)
            nc.sync.dma_start(out=whn_s[:], in_=whn[:])
            bihn_s = pers.tile([64, 1], F32)
            nc.sync.dma_start(out=bihn_s[:], in_=bihn[:])
            BC1_s = pers.tile([64, 8], BF16)
            nc.sync.dma_start(out=BC1_s[:], in_=BC1[:])
            W12A_s = pers.tile([128, 128], BF16)
            nc.sync.dma_start(out=W12A_s[:], in_=W12A[:])
            W12B_s = pers.tile([128, 128], BF16)
            nc.sync.dma_start(out=W12B_s[:], in_=W12B[:])
            b1b_s = pers.tile([128, 256], BF16)
            nc.sync.dma_start(out=b1b_s[:], in_=b1b[:])
            W2s_s = pers.tile([128, 128], BF16)
            nc.sync.dma_start(out=W2s_s[:], in_=W2s[:])
            BC2_s = pers.tile([128, 4], BF16)
            nc.sync.dma_start(out=BC2_s[:], in_=BC2[:])
            b2b_s = pers.tile([128, 64], BF16)
            nc.sync.dma_start(out=b2b_s[:], in_=b2b[:])
            fcw_s = pers.tile([64, 10], F32)
            nc.sync.dma_start(out=fcw_s[:], in_=fcw[:])
            fcbb_s = pers.tile([128, 10], F32)
            nc.sync.dma_start(out=fcbb_s[:], in_=fcbb[:])
            idx_sb = pers.tile([128, TOT_SLOT // 16], I16)
            nc.sync.dma_start(out=idx_sb[:], in_=idx_wr[:])
            drel_sb = pers.tile([128, TOT_TILES], U8)
            nc.sync.dma_start(out=drel_sb[:], in_=drel_pt[:])
            batch_sb = pers.tile([128, NW], F32)
            nc.sync.dma_start(out=batch_sb[:], in_=batch_wd[:])
            adt1_sb = pers.tile([128, NW, 4], BF16)
            adt2_sb = pers.tile([128, NW, 1], BF16)

            # ---- phase 1: GRU -> xcat1l (h|asrc), adt1_sb ----
            with (
                tc.tile_pool(name="gx", bufs=2) as gx,
                tc.tile_pool(name="gh", bufs=2) as gh,
                tc.tile_pool(name="gv", bufs=3) as gv,
                tc.tile_pool(name="gp1", bufs=1, space="PSUM") as gp1,
                tc.tile_pool(name="gp2", bufs=1, space="PSUM") as gp2,
                tc.tile_pool(name="gp3", bufs=1, space="PSUM") as gp3,
                tc.tile_pool(name="gp4", bufs=2, space="PSUM") as gp4,
            ):
                for it in range(NT):
                    xt8 = gx.tile([128, T, GT], BF16, tag="xt8")
                    for t in range(T):
                        nc.sync.dma_start(out=xt8[:, t, :],
                                          in_=xpT[t, :, it * GT:(it + 1) * GT])
                    hT = gh.tile([65, GT], BF16, tag="hT")
                    nc.vector.memset(hT[0:64, :], 0.0)
                    nc.vector.memset(hT[64:65, :], 1.0)
                    for t in range(T):
                        prz = gp1.tile([64, 2 * GT], F32, tag="prz")
                        nc.tensor.matmul(out=prz[:, 0:GT], lhsT=wihT_s[:, 0:64],
                                         rhs=xt8[:, t, :], start=True, stop=False)
                        nc.tensor.matmul(out=prz[:, 0:GT], lhsT=whrz_s[:, 0:64],
                                         rhs=hT[:], start=False, stop=True)
                        nc.tensor.matmul(out=prz[:, GT:2 * GT], lhsT=wihT_s[:, 64:128],
                                         rhs=xt8[:, t, :], start=True, stop=False)
                        nc.tensor.matmul(out=prz[:, GT:2 * GT], lhsT=whrz_s[:, 64:128],
                                         rhs=hT[:], start=False, stop=True)
                        pin = gp2.tile([64, GT], F32, tag="pin")
                        nc.tensor.matmul(out=pin[:], lhsT=wihT_s[:, 128:192],
                                         rhs=xt8[:, t, :], start=True, stop=True)
                        phn = gp3.tile([64, GT], F32, tag="phn")
                        nc.tensor.matmul(out=phn[:], lhsT=whn_s[:], rhs=hT[:],
                                         start=True, stop=True)
                        rz = gv.tile([64, 2 * GT], BF16, tag="rz")
                        nc.scalar.activation(out=rz[:], in_=prz[:], func=AF.Sigmoid)
                        tmp = gv.tile([64, GT], BF16, tag="tmp")
                        nc.vector.tensor_mul(out=tmp[:], in0=rz[:, 0:GT], in1=phn[:])
                        t3 = gv.tile([64, GT], F32, tag="t3")
                        nc.vector.tensor_add(out=t3[:], in0=pin[:], in1=tmp[:])
                        nh = gv.tile([64, GT], BF16, tag="nh")
                        nc.scalar.activation(out=nh[:], in_=t3[:], func=AF.Tanh,
                                             bias=bihn_s[:, 0:1])
                        s1 = gv.tile([64, GT], BF16, tag="s1")
                        nc.vector.tensor_sub(out=s1[:], in0=hT[0:64, :], in1=nh[:])
                        s2 = gv.tile([64, GT], BF16, tag="s2")
                        nc.vector.tensor_mul(out=s2[:], in0=rz[:, GT:2 * GT], in1=s1[:])
                        nc.vector.tensor_add(out=hT[0:64, :], in0=nh[:], in1=s2[:])
                    for cc in range(GT // 128):
                        w = it * (GT // 128) + cc
                        pt = gp4.tile([128, 64], BF16, tag="pt")
                        nc.tensor.transpose(out=pt[:],
                                            in_=hT[0:64, cc * 128:(cc + 1) * 128],
                                            identity=identb[0:64, 0:64])
                        pat = gp4.tile([128, 8], F32, tag="pat")
                        nc.tensor.matmul(out=pat[:], lhsT=hT[0:64, cc * 128:(cc + 1) * 128],
                                         rhs=BC1_s[:], start=True, stop=True)
                        xc = gv.tile([128, 128], BF16, tag="xc")
                        nc.scalar.copy(out=xc[:, 0:64], in_=pt[:])
                        nc.vector.tensor_copy(out=xc[:, 64:68], in_=pat[:, 0:4])
                        nc.vector.memset(xc[:, 68:128], 0.0)
                        nc.vector.tensor_copy(out=adt1_sb[:, w, :], in_=pat[:, 4:8])
                        nc.sync.dma_start(out=xcat1l[w * 128:(w + 1) * 128, :], in_=xc[:])

            # ---- AllGather table1, split into shards ----
            tc.strict_bb_all_engine_barrier()
            nc.gpsimd.collective_compute(
                "AllGather", OP.bypass, replica_groups=rg_all,
                ins=[xcat1l[:]], outs=[table1[:]])
            for s in range(NSHARD):
                nc.sync.dma_start(out=tb1s[s][:], in_=table1[s * SH:(s + 1) * SH, :])
            tc.strict_bb_all_engine_barrier()

            # ---- GAT layer over windows (shared for layer 1 / layer 2) ----
            def gat_windows(tbls, adt_sb, nheads, payw, post_fn):
                """payw: scatter matmul width (4+256 for L1, 1+64 for L2)."""
                with (
                    tc.tile_pool(name="pg", bufs=3) as pg,
                    tc.tile_pool(name="pS", bufs=2) as pS,
                    tc.tile_pool(name="pd", bufs=2) as pd,
                    tc.tile_pool(name="pu", bufs=2) as pu,
                    tc.tile_pool(name="pM", bufs=2) as pM,
                    tc.tile_pool(name="pw", bufs=2) as pw,
                    tc.tile_pool(name="ps2", bufs=2, space="PSUM") as ps2,
                    tc.tile_pool(name="psA", bufs=1, space="PSUM") as psA,
                    tc.tile_pool(name="psB", bufs=1, space="PSUM") as psB,
                ):
                    for w in range(NW):
                        pblk = ps2.tile([128, payw], F32, tag="pblk")
                        first = True
                        for s in range(NSHARD):
                            b = w * NSHARD + s
                            Tb = int(tiles[b])
                            t0 = int(tile_off[b])
                            g = pg.tile([128, Tb, 128], BF16, tag=f"g{Tb}")
                            for q0 in range(0, Tb, 8):
                                qn = min(8, Tb - q0)
                                nc.gpsimd.dma_gather(
                                    out_ap=g[:, q0:q0 + qn, :], in_ap=tbls[s][:],
                                    idxs_ap=idx_sb[:, (t0 + q0) * 8:(t0 + q0 + qn) * 8],
                                    num_idxs=qn * 128, num_idxs_reg=qn * 128,
                                    elem_size=128)
                            S = pS.tile([128, Tb, 128], BF16, tag=f"S{Tb}")
                            nc.vector.tensor_tensor(
                                out=S[:],
                                in0=iota_row[:].unsqueeze(1).broadcast_to([128, Tb, 128]),
                                in1=drel_sb[:, t0:t0 + Tb].unsqueeze(2).broadcast_to([128, Tb, 128]),
                                op=OP.is_equal)
                            drT = pd.tile([128, Tb, 128], U8, tag=f"dT{Tb}")
                            nc.sync.dma_start(
                                out=drT[:],
                                in_=drelT[0:1, t0:t0 + Tb, :].partition_broadcast(128))
                            Sd = pS.tile([128, Tb, 128], BF16, tag=f"Sd{Tb}")
                            nc.vector.tensor_scalar(
                                out=Sd[:], in0=drT[:], scalar1=iota_p[:, 0:1],
                                scalar2=None, op0=OP.is_equal)
                            padp = psA.tile([128, Tb * nheads], F32, tag="padp")
                            for t in range(Tb):
                                nc.tensor.matmul(
                                    out=padp[:, t * nheads:(t + 1) * nheads],
                                    lhsT=Sd[:, t, :], rhs=adt_sb[:, w, :],
                                    start=True, stop=True)
                            u = pu.tile([128, Tb, nheads], F32, tag=f"u{Tb}")
                            nc.vector.tensor_add(
                                out=u[:], in0=g[:, :, 64:64 + nheads],
                                in1=padp[:].rearrange("p (t c) -> p t c", t=Tb))
                            e1 = pu.tile([128, Tb, nheads], BF16, tag=f"e1{Tb}")
                            nc.scalar.activation(out=e1[:], in_=u[:], func=AF.Exp)
                            e2 = pu.tile([128, Tb, nheads], BF16, tag=f"e2{Tb}")
                            nc.scalar.activation(out=e2[:], in_=u[:], func=AF.Exp, scale=0.2)
                            ee = pu.tile([128, Tb, nheads], BF16, tag=f"ee{Tb}")
                            nc.vector.tensor_tensor(out=ee[:], in0=e1[:], in1=e2[:], op=OP.max)
                            M = pM.tile([128, Tb, payw], BF16, tag=f"M{Tb}")
                            nc.vector.tensor_copy(out=M[:, :, 0:nheads], in_=ee[:])
                            if nheads == 4:
                                nc.vector.tensor_tensor(
                                    out=M[:, :, 4:260].rearrange("p t (h c) -> p t h c", h=4),
                                    in0=g[:, :, 0:64].unsqueeze(2).broadcast_to([128, Tb, 4, 64]),
                                    in1=ee[:].unsqueeze(3).broadcast_to([128, Tb, 4, 64]),
                                    op=OP.mult)
                            else:
                                nc.vector.tensor_tensor(
                                    out=M[:, :, 1:65],
                                    in0=g[:, :, 0:64],
                                    in1=ee[:].to_broadcast([128, Tb, 64]),
                                    op=OP.mult)
                            for t in range(Tb):
                                nc.tensor.matmul(
                                    out=pblk[:], lhsT=S[:, t, :], rhs=M[:, t, :],
                                    start=first, stop=(s == NSHARD - 1 and t == Tb - 1))
                                first = False
                        post_fn(w, pblk, pw, psB)

            # ---- layer 1 post: h1 = relu(z/denom @ W1 + b1) -> xcat2, adt2 ----
            def post1(w, pblk, pw, psB):
                dn = pw.tile([128, 4], F32, tag="dn")
                nc.vector.tensor_scalar(out=dn[:], in0=pblk[:, 0:4], scalar1=1e-12,
                                        scalar2=None, op0=OP.max)
                rec = pw.tile([128, 4], F32, tag="rec")
                nc.vector.reciprocal(out=rec[:], in_=dn[:])
                zn = pw.tile([128, 256], BF16, tag="zn")
                for h in range(4):
                    nc.vector.tensor_scalar(
                        out=zn[:, 64 * h:64 * (h + 1)],
                        in0=pblk[:, 4 + 64 * h:4 + 64 * (h + 1)],
                        scalar1=rec[:, h:h + 1], scalar2=None, op0=OP.mult)
                tpz = pw.tile([128, 256], BF16, tag="tpz")
                for half in range(2):
                    ptp = psB.tile([128, 128], BF16, tag="ptp")
                    nc.tensor.transpose(out=ptp[:], in_=zn[:, 128 * half:128 * (half + 1)],
                                        identity=identb[:])
                    nc.scalar.copy(out=tpz[:, 128 * half:128 * (half + 1)], in_=ptp[:])
                h1p = psB.tile([128, 256], F32, tag="h1p")
                nc.tensor.matmul(out=h1p[:, 0:128], lhsT=tpz[:, 0:128], rhs=W12A_s[:],
                                 start=True, stop=True)
                nc.tensor.matmul(out=h1p[:, 128:256], lhsT=tpz[:, 128:256], rhs=W12B_s[:],
                                 start=True, stop=True)
                h1 = pw.tile([128, 256], BF16, tag="h1")
                nc.vector.tensor_add(out=h1[:], in0=h1p[:], in1=b1b_s[:])
                nc.vector.tensor_scalar(out=h1[:], in0=h1[:], scalar1=0.0,
                                        scalar2=None, op0=OP.max)
                th1 = pw.tile([128, 256], BF16, tag="th1")
                for half in range(2):
                    ptp = psB.tile([128, 128], BF16, tag="ptp")
                    nc.tensor.transpose(out=ptp[:], in_=h1[:, 128 * half:128 * (half + 1)],
                                        identity=identb[:])
                    nc.scalar.copy(out=th1[:, 128 * half:128 * (half + 1)], in_=ptp[:])
                xsc = psB.tile([128, 66], F32, tag="xsc")
                xs2p = xsc[:, 0:64]
                pat2 = xsc[:, 64:66]
                for half in range(2):
                    nc.tensor.matmul(out=xs2p, lhsT=th1[:, 128 * half:128 * (half + 1)],
                                     rhs=W2s_s[:, 64 * half:64 * (half + 1)],
                                     start=(half == 0), stop=(half == 1))
                    nc.tensor.matmul(out=pat2, lhsT=th1[:, 128 * half:128 * (half + 1)],
                                     rhs=BC2_s[:, 2 * half:2 * half + 2],
                                     start=(half == 0), stop=(half == 1))
                xc2 = pw.tile([128, 128], BF16, tag="xc2")
                nc.scalar.copy(out=xc2[:, 0:64], in_=xs2p)
                nc.vector.tensor_copy(out=xc2[:, 64:65], in_=pat2[:, 0:1])
                nc.vector.memset(xc2[:, 65:128], 0.0)
                nc.vector.tensor_copy(out=adt2_sb[:, w, :], in_=pat2[:, 1:2])
                nc.sync.dma_start(out=xcat2l[w * 128:(w + 1) * 128, :], in_=xc2[:])

            gat_windows(tb1s, adt1_sb, 4, 260, post1)

            tc.strict_bb_all_engine_barrier()
            nc.gpsimd.collective_compute(
                "AllGather", OP.bypass, replica_groups=rg_all,
                ins=[xcat2l[:]], outs=[table2[:]])
            for s in range(NSHARD):
                nc.sync.dma_start(out=tb2s[s][:], in_=table2[s * SH:(s + 1) * SH, :])
            tc.strict_bb_all_engine_barrier()

            # ---- layer 2 post: pooling into ppool psum ----
            pp_ctx = tc.tile_pool(name="psPool", bufs=1, space="PSUM")
            psPool = pp_ctx.__enter__()
            ppool = psPool.tile([128, 65], F32)

            def post2(w, pblk, pw, psB):
                dn2 = pw.tile([128, 1], F32, tag="dn2")
                nc.vector.tensor_scalar(out=dn2[:], in0=pblk[:, 0:1], scalar1=1e-12,
                                        scalar2=None, op0=OP.max)
                rec2 = pw.tile([128, 1], F32, tag="rec2")
                nc.vector.reciprocal(out=rec2[:], in_=dn2[:])
                ph = pw.tile([128, 65], BF16, tag="ph")
                nc.vector.tensor_scalar(out=ph[:, 0:64], in0=pblk[:, 1:65],
                                        scalar1=rec2[:, 0:1], scalar2=None, op0=OP.mult)
                nc.vector.tensor_add(out=ph[:, 0:64], in0=ph[:, 0:64], in1=b2b_s[:])
                nc.vector.tensor_scalar(out=ph[:, 0:64], in0=ph[:, 0:64], scalar1=0.0,
                                        scalar2=None, op0=OP.max)
                nc.vector.memset(ph[:, 64:65], 1.0)
                Sb = pw.tile([128, 128], BF16, tag="Sb")
                nc.vector.tensor_scalar(out=Sb[:], in0=iota_row[:],
                                        scalar1=batch_sb[:, w:w + 1], scalar2=None,
                                        op0=OP.is_equal)
                nc.tensor.matmul(out=ppool[:], lhsT=Sb[:], rhs=ph[:],
                                 start=(w == 0), stop=(w == NW - 1))

            gat_windows(tb2s, adt2_sb, 1, 65, post2)

            # ---- tail: AllReduce pools, fc, log_softmax ----
            with tc.tile_pool(name="rpre", bufs=1) as rpre:
                pr = rpre.tile([128, 65], F32)
                nc.scalar.copy(out=pr[:], in_=ppool[:])
                nc.sync.dma_start(out=arin[:], in_=pr[:])
            pp_ctx.__exit__(None, None, None)
            with (
                tc.tile_pool(name="r5", bufs=1) as r5,
                tc.tile_pool(name="r5q", bufs=1, space="PSUM") as r5q,
            ):
                tc.strict_bb_all_engine_barrier()
                nc.gpsimd.collective_compute(
                    "AllReduce", OP.add, replica_groups=rg_all,
                    ins=[arin[:]], outs=[arout[:]])
                tc.strict_bb_all_engine_barrier()
                ar = r5.tile([128, 65], F32)
                nc.sync.dma_start(out=ar[:], in_=arout[:])
                cm = r5.tile([128, 1], F32)
                nc.vector.tensor_scalar(out=cm[:], in0=ar[:, 64:65], scalar1=1.0,
                                        scalar2=None, op0=OP.max)
                cr = r5.tile([128, 1], F32)
                nc.vector.reciprocal(out=cr[:], in_=cm[:])
                gf = r5.tile([128, 64], F32)
                nc.vector.tensor_scalar(out=gf[:], in0=ar[:, 0:64], scalar1=cr[:, 0:1],
                                        scalar2=None, op0=OP.mult)
                identf = r5.tile([128, 128], F32)
                make_identity(nc, identf[:])
                pgt = r5q.tile([64, 128], F32)
                nc.tensor.transpose(out=pgt[:], in_=gf[:], identity=identf[:])
                gfT = r5.tile([64, 128], F32)
                nc.scalar.copy(out=gfT[:], in_=pgt[:])
                plg = r5q.tile([128, 10], F32)
                nc.tensor.matmul(out=plg[:], lhsT=gfT[:], rhs=fcw_s[:], start=True, stop=True)
                lg = r5.tile([128, 16], F32)
                nc.vector.tensor_add(out=lg[:, 0:10], in0=plg[:], in1=fcbb_s[:])
                mx = r5.tile([128, 1], F32)
                nc.vector.reduce_max(out=mx[:], in_=lg[:, 0:10], axis=mybir.AxisListType.X)
                tsh = r5.tile([128, 16], F32)
                nc.vector.tensor_scalar(out=tsh[:, 0:10], in0=lg[:, 0:10],
                                        scalar1=mx[:, 0:1], scalar2=None, op0=OP.subtract)
                exs = r5.tile([128, 16], F32)
                se = r5.tile([128, 1], F32)
                nc.scalar.activation(out=exs[:, 0:10], in_=tsh[:, 0:10], func=AF.Exp,
                                     accum_out=se[:])
                ln = r5.tile([128, 1], F32)
                nc.scalar.activation(out=ln[:], in_=se[:], func=AF.Ln)
                res = r5.tile([128, 16], F32)
                nc.vector.memset(res[:], 0.0)
                nc.vector.tensor_scalar(out=res[:, 0:10], in0=tsh[:, 0:10],
                                        scalar1=ln[:, 0:1], scalar2=None, op0=OP.subtract)
                nc.sync.dma_start(out=out[:], in_=res[:])
    nc.compile()
    return nc


# ---------------- self-contained entry point ----------------
_CACHE = {}


def kernel(**inputs):
    """Full DAGNN forward. Takes the unsharded inputs from setup_inputs();
    returns log-softmax output [num_graphs, 10] float32."""
    x = np.asarray(inputs["x"], np.float32)
    edge_index = np.asarray(inputs["edge_index"])
    batch = np.asarray(inputs["batch"])
    G = int(inputs["num_graphs"])
    weights = [np.asarray(inputs[k], np.float32) for k in (
        "gru_w_ih", "gru_w_hh", "gru_b_ih", "gru_b_hh",
        "W1", "att_src1", "att_dst1", "b1",
        "W2", "att_src2", "att_dst2", "b2", "fc_w", "fc_b")]
    N = x.shape[0]
    E = edge_index.shape[1]
    P = 8

    from concourse.bass_utils import run_bass_kernel_spmd
    cfg = Cfg(N, E, G, P)
    per_core = host_prep(cfg, edge_index, batch)
    in_maps = build_inputs(cfg, x, weights, per_core)
    key = (N, E, G, P, cfg.NPAD2, cfg.TOT_TILES, tuple(cfg.tiles[:8]))
    if key not in _CACHE:
        _CACHE[key] = build_kernel(cfg)
    nc = _CACHE[key]
    res = run_bass_kernel_spmd(nc, in_maps, core_ids=list(range(P)))
    out = np.asarray(res.results[0]["out"][:G, :10], np.float32)
    return out


# revision 13
# speedup vs baseline: 2.5976x; 1.0068x over previous
"""DAGNN (GRU + 2xGAT + mean-pool + fc + log_softmax) on 8 TRN2 cores via Bass/Tile.

Sharding: nodes split across cores by dst-range (edges sorted by dst, split at
dst boundaries), so each core's GRU computes exactly the h/attention values its
GAT dst windows need locally. Edge payload gathers use batched dma_gather from
a 4-way row-sharded bf16 table (int16 index limit); per-window dst attention
terms are expanded on-chip via selection-matrix matmuls. Feature tables are
AllGathered; graph pooling partial sums are AllReduced.
"""
import sys
import numpy as np

sys.path.insert(0, "/opt/trn_rl_repo")

import ml_dtypes
import concourse.bass as bass
import concourse.bacc as bacc
import concourse.mybir as mybir
import concourse.tile as tile
from concourse.masks import make_identity

F32 = mybir.dt.float32
BF16 = mybir.dt.bfloat16
I16 = mybir.dt.int16
U8 = mybir.dt.uint8
AF = mybir.ActivationFunctionType
OP = mybir.AluOpType

NSH = 4          # table row shards (int16 gather index limit)
GT = 512         # GRU node tile


def _ceil(a, b):
    return -(-a // b)


class Cfg:
    def __init__(self, N, E, G, P):
        self.N, self.E, self.G, self.P = N, E, G, P
        self.T, self.D, self.H = 8, 128, 64
        self.HEADS, self.C1, self.C2 = 4, 256, 64


def host_prep(cfg, edge_index, batch):
    N, E, P = cfg.N, cfg.E, cfg.P
    src = np.concatenate([np.asarray(edge_index[0], np.int64), np.arange(N, dtype=np.int64)])
    dst = np.concatenate([np.asarray(edge_index[1], np.int64), np.arange(N, dtype=np.int64)])
    order = np.argsort(dst, kind="stable")
    ss, dd = src[order], dst[order]
    Etot = ss.shape[0]

    bounds = [0]
    for k in range(1, P):
        pos = (k * Etot) // P
        while pos < Etot and dd[pos] == dd[pos - 1]:
            pos += 1
        bounds.append(pos)
    bounds.append(Etot)
    n0 = np.zeros(P + 1, np.int64)
    n0[P] = N
    for c in range(1, P):
        n0[c] = dd[bounds[c]]
    ranges = np.diff(n0)
    NPAD2 = _ceil(int(ranges.max()), GT) * GT
    NW = NPAD2 // 128
    SH = (P * NPAD2) // NSH
    assert SH - 1 <= 32767, f"shard too large for int16: {SH}"
    cfg.n0, cfg.NPAD2, cfg.NW, cfg.SH = n0, NPAD2, NW, SH
    cfg.NT = NPAD2 // GT

    owner = np.searchsorted(n0[1:P], np.arange(N), side="right")
    g2r = owner * NPAD2 + (np.arange(N) - n0[owner])
    shard_of = (g2r // SH).astype(np.int64)
    rel_of = (g2r % SH).astype(np.int16)

    # pass 1: per-(window, shard) edge counts per core -> uniform tile counts
    NB = NW * NSH
    kws = np.zeros((P, NB), np.int64)
    per_edges = []
    for c in range(P):
        sl = slice(bounds[c], bounds[c + 1])
        ssc, ddc = ss[sl], dd[sl]
        w_arr = (ddc - n0[c]) // 128
        s_arr = shard_of[ssc]
        key = (w_arr * NSH + s_arr).astype(np.int64)
        kws[c] = np.bincount(key, minlength=NB)
        per_edges.append((ssc, ddc, w_arr, key))
    tiles = np.maximum(1, _ceil(kws.max(axis=0), 128)).astype(np.int64)
    tile_off = np.concatenate([[0], np.cumsum(tiles)])
    TOT_TILES = int(tile_off[-1])
    cfg.tiles, cfg.tile_off, cfg.TOT_TILES = tiles, tile_off, TOT_TILES
    cfg.TBMAX = int(tiles.max())

    per_core = []
    for c in range(P):
        ssc, ddc, w_arr, key = per_edges[c]
        order2 = np.argsort(key, kind="stable")
        sk = key[order2]
        grp_start = np.searchsorted(sk, np.arange(NB))
        rank = np.arange(sk.shape[0]) - grp_start[sk]
        slotpos = tile_off[sk] * 128 + rank
        TOT_SLOT = TOT_TILES * 128
        srel = np.zeros(TOT_SLOT, np.int16)
        drel = np.full(TOT_SLOT, 255, np.uint8)
        srel[slotpos] = rel_of[ssc[order2]]
        drel[slotpos] = (ddc[order2] - n0[c] - 128 * w_arr[order2]).astype(np.uint8)
        # wrapped gather indices: idx i of a (tile-aligned) run at [i%16, i//16]
        wr = np.ascontiguousarray(srel.reshape(TOT_SLOT // 16, 16).T)
        idx_wr = np.tile(wr, (8, 1))                                   # [128, TOT_SLOT//16]
        drel_pt = np.ascontiguousarray(drel.reshape(TOT_TILES, 128).T)  # [128, TOT_TILES]
        drelT = drel.reshape(1, TOT_TILES, 128).copy()                  # [1, TOT_TILES, 128]
        bd = np.full(NPAD2, 999.0, np.float32)
        rg = int(ranges[c])
        bd[:rg] = np.asarray(batch, np.int64)[n0[c]:n0[c + 1]].astype(np.float32)
        batch_wd = np.ascontiguousarray(bd.reshape(NW, 128).T)          # [128, NW]
        per_core.append(dict(idx_wr=idx_wr, drel_pt=drel_pt, drelT=drelT,
                             batch_wd=batch_wd, rg=rg))
    return per_core


def build_inputs(cfg, x, weights, per_core):
    (gru_w_ih, gru_w_hh, gru_b_ih, gru_b_hh, W1, att_src1, att_dst1, b1,
     W2, att_src2, att_dst2, b2, fc_w, fc_b) = weights
    P, NPAD2 = cfg.P, cfg.NPAD2
    bf = ml_dtypes.bfloat16

    # BC1 [64, 8]: cols 0:4 src-att coeffs per head, 4:8 dst-att
    BC1 = np.zeros((64, 8), np.float32)
    for h in range(4):
        Wh = W1[:, 64 * h:64 * (h + 1)]
        BC1[:, h] = Wh @ att_src1[h]
        BC1[:, 4 + h] = Wh @ att_dst1[h]
    # W1 block-diagonal pairs for transposed apply
    W12A = np.zeros((128, 128), np.float32)
    W12B = np.zeros((128, 128), np.float32)
    W12A[0:64, 0:64] = W1[:, 0:64]
    W12A[64:128, 64:128] = W1[:, 64:128]
    W12B[0:64, 0:64] = W1[:, 128:192]
    W12B[64:128, 64:128] = W1[:, 192:256]
    # W2 halves side by side; BC2 [128, 4]: cols 2h = [src|dst] coeffs, half h
    W2s = np.zeros((128, 128), np.float32)
    W2s[:, 0:64] = W2[0:128, :]
    W2s[:, 64:128] = W2[128:256, :]
    a2 = W2 @ att_src2[0]   # [256]
    d2 = W2 @ att_dst2[0]
    BC2 = np.zeros((128, 4), np.float32)
    BC2[:, 0] = a2[0:128]
    BC2[:, 1] = d2[0:128]
    BC2[:, 2] = a2[128:256]
    BC2[:, 3] = d2[128:256]

    com = dict(
        wihT=np.ascontiguousarray(gru_w_ih.T).astype(bf),               # [128,192]
        whrz=np.concatenate([gru_w_hh.T[:, 0:128],
                             (gru_b_ih + gru_b_hh)[None, 0:128]], 0).astype(bf),  # [65,128]
        whn=np.concatenate([gru_w_hh.T[:, 128:192],
                            gru_b_hh[None, 128:192]], 0).astype(bf),    # [65,64]
        bihn=np.ascontiguousarray(gru_b_ih[128:192].reshape(64, 1)).astype(np.float32),
        BC1=BC1.astype(bf),
        W12A=W12A.astype(bf), W12B=W12B.astype(bf),
        b1b=np.broadcast_to(b1, (128, 256)).astype(bf).copy(),
        W2s=W2s.astype(bf), BC2=BC2.astype(bf),
        b2b=np.broadcast_to(b2, (128, 64)).astype(bf).copy(),
        fcw=fc_w.astype(np.float32),
        fcbb=np.broadcast_to(fc_b, (128, 10)).astype(np.float32).copy(),
    )
    in_maps = []
    for c in range(P):
        pc = per_core[c]
        rg = pc["rg"]
        xp = np.zeros((NPAD2, cfg.T, cfg.D), np.float32)
        xp[:rg] = x[cfg.n0[c]:cfg.n0[c + 1]]
        xpT = np.ascontiguousarray(xp.transpose(1, 2, 0)).astype(bf)     # [8,128,NPAD2]
        m = dict(com)
        m.update(xpT=xpT, idx_wr=pc["idx_wr"], drel_pt=pc["drel_pt"],
                 drelT=pc["drelT"], batch_wd=pc["batch_wd"])
        in_maps.append(m)
    return in_maps


def build_kernel(cfg, dbg=False):
    P, T, NPAD2, NW, SH = cfg.P, cfg.T, cfg.NPAD2, cfg.NW, cfg.SH
    NT, NSHARD = cfg.NT, NSH
    tiles, tile_off, TOT_TILES = cfg.tiles, cfg.tile_off, cfg.TOT_TILES
    TOT_SLOT = TOT_TILES * 128
    rg_all = [list(range(P))]

    nc = bacc.Bacc("TRN2", target_bir_lowering=False, debug=False,
                   dynamic_dma_scratch_size=32768)
    # inputs
    xpT = nc.dram_tensor("xpT", [T, 128, NPAD2], BF16, kind="ExternalInput")
    wihT = nc.dram_tensor("wihT", [128, 192], BF16, kind="ExternalInput")
    whrz = nc.dram_tensor("whrz", [65, 128], BF16, kind="ExternalInput")
    whn = nc.dram_tensor("whn", [65, 64], BF16, kind="ExternalInput")
    bihn = nc.dram_tensor("bihn", [64, 1], F32, kind="ExternalInput")
    BC1 = nc.dram_tensor("BC1", [64, 8], BF16, kind="ExternalInput")
    W12A = nc.dram_tensor("W12A", [128, 128], BF16, kind="ExternalInput")
    W12B = nc.dram_tensor("W12B", [128, 128], BF16, kind="ExternalInput")
    b1b = nc.dram_tensor("b1b", [128, 256], BF16, kind="ExternalInput")
    W2s = nc.dram_tensor("W2s", [128, 128], BF16, kind="ExternalInput")
    BC2 = nc.dram_tensor("BC2", [128, 4], BF16, kind="ExternalInput")
    b2b = nc.dram_tensor("b2b", [128, 64], BF16, kind="ExternalInput")
    fcw = nc.dram_tensor("fcw", [64, 10], F32, kind="ExternalInput")
    fcbb = nc.dram_tensor("fcbb", [128, 10], F32, kind="ExternalInput")
    idx_wr = nc.dram_tensor("idx_wr", [128, TOT_SLOT // 16], I16, kind="ExternalInput")
    drel_pt = nc.dram_tensor("drel_pt", [128, TOT_TILES], U8, kind="ExternalInput")
    drelT = nc.dram_tensor("drelT", [1, TOT_TILES, 128], U8, kind="ExternalInput")
    batch_wd = nc.dram_tensor("batch_wd", [128, NW], F32, kind="ExternalInput")
    out = nc.dram_tensor("out", [128, 16], F32, kind="ExternalOutput")
    # internal dram
    xcat1l = nc.dram_tensor("xcat1l", [NPAD2, 128], BF16)
    table1 = nc.dram_tensor("table1", [P * NPAD2, 128], BF16, addr_space="Shared")
    tb1s = [nc.dram_tensor(f"tb1s{s}", [SH, 128], BF16) for s in range(NSHARD)]
    xcat2l = nc.dram_tensor("xcat2l", [NPAD2, 128], BF16)
    table2 = nc.dram_tensor("table2", [P * NPAD2, 128], BF16, addr_space="Shared")
    tb2s = [nc.dram_tensor(f"tb2s{s}", [SH, 128], BF16) for s in range(NSHARD)]
    arin = nc.dram_tensor("arin", [128, 65], F32)
    arout = nc.dram_tensor("arout", [128, 65], F32, addr_space="Shared")

    with tile.TileContext(nc) as tc:
        with tc.tile_pool(name="pers", bufs=1) as pers:
            # ---- persistent: weights, indices, iotas ----
            identb = pers.tile([128, 128], BF16)
            make_identity(nc, identb[:])
            iota_row = pers.tile([128, 128], U8)
            nc.gpsimd.iota(iota_row[:], pattern=[[1, 128]], base=0, channel_multiplier=0,
                           allow_small_or_imprecise_dtypes=True)
            iota_p = pers.tile([128, 1], F32)
            nc.gpsimd.iota(iota_p[:], pattern=[[0, 1]], base=0, channel_multiplier=1,
                           allow_small_or_imprecise_dtypes=True)
            wihT_s = pers.tile([128, 192], BF16)
            nc.sync.dma_start(out=wihT_s[:], in_=wihT[:])
            whrz_s = pers.tile([65, 128], BF16)
            nc.sync.dma_start(out=whrz_s[:], in_=whrz[:])
            whn_s = pers.tile([65, 64], BF16)
            nc.sync.dma_start(out=whn_s[:], in_=whn[:])
            bihn_s = pers.tile([64, 1], F32)
            nc.sync.dma_start(out=bihn_s[:], in_=bihn[:])
            BC1_s = pers.tile([64, 8], BF16)
            nc.sync.dma_start(out=BC1_s[:], in_=BC1[:])
            W12A_s = pers.tile([128, 128], BF16)
            nc.sync.dma_start(out=W12A_s[:], in_=W12A[:])
            W12B_s = pers.tile([128, 128], BF16)
            nc.sync.dma_start(out=W12B_s[:], in_=W12B[:])
            b1b_s = pers.tile([128, 256], BF16)
            nc.sync.dma_start(out=b1b_s[:], in_=b1b[:])
            W2s_s = pers.tile([128, 128], BF16)
            nc.sync.dma_start(out=W2s_s[:], in_=W2s[:])
            BC2_s = pers.tile([128, 4], BF16)
            nc.sync.dma_start(out=BC2_s[:], in_=BC2[:])
            b2b_s = pers.tile([128, 64], BF16)
            nc.sync.dma_start(out=b2b_s[:], in_=b2b[:])
            fcw_s = pers.tile([64, 10], F32)
            nc.sync.dma_start(out=fcw_s[:], in_=fcw[:])
            fcbb_s = pers.tile([128, 10], F32)
            nc.sync.dma_start(out=fcbb_s[:], in_=fcbb[:])
            idx_sb = pers.tile([128, TOT_SLOT // 16], I16)
            nc.sync.dma_start(out=idx_sb[:], in_=idx_wr[:])
            drel_sb = pers.tile([128, TOT_TILES], U8)
            nc.sync.dma_start(out=drel_sb[:], in_=drel_pt[:])
            batch_sb = pers.tile([128, NW], F32)
            nc.sync.dma_start(out=batch_sb[:], in_=batch_wd[:])
            adt1_sb = pers.tile([128, NW, 4], BF16)
            adt2_sb = pers.tile([128, NW, 1], BF16)
            nidx_regs = {}
            for b in range(NW * NSHARD):
                Tb = int(tiles[b])
                for q0 in range(0, Tb, 8):
                    n = min(8, Tb - q0) * 128
                    if n not in nidx_regs:
                        nidx_regs[n] = nc.gpsimd.to_reg(n)

            # ---- phase 1: GRU -> xcat1l (h|asrc), adt1_sb ----
            with (
                tc.tile_pool(name="gx", bufs=2) as gx,
                tc.tile_pool(name="gh", bufs=2) as gh,
                tc.tile_pool(name="gv", bufs=3) as gv,
                tc.tile_pool(name="gp1", bufs=1, space="PSUM") as gp1,
                tc.tile_pool(name="gp2", bufs=1, space="PSUM") as gp2,
                tc.tile_pool(name="gp3", bufs=1, space="PSUM") as gp3,
                tc.tile_pool(name="gp4", bufs=2, space="PSUM") as gp4,
            ):
                for it in range(NT):
                    xt8 = gx.tile([128, T, GT], BF16, tag="xt8")
                    for t in range(T):
                        nc.sync.dma_start(out=xt8[:, t, :],
                                          in_=xpT[t, :, it * GT:(it + 1) * GT])
                    hT = gh.tile([65, GT], BF16, tag="hT")
                    nc.vector.memset(hT[0:64, :], 0.0)
                    nc.vector.memset(hT[64:65, :], 1.0)
                    for t in range(T):
                        prz = gp1.tile([64, 2 * GT], F32, tag="prz")
                        nc.tensor.matmul(out=prz[:, 0:GT], lhsT=wihT_s[:, 0:64],
                                         rhs=xt8[:, t, :], start=True, stop=False)
                        nc.tensor.matmul(out=prz[:, 0:GT], lhsT=whrz_s[:, 0:64],
                                         rhs=hT[:], start=False, stop=True)
                        nc.tensor.matmul(out=prz[:, GT:2 * GT], lhsT=wihT_s[:, 64:128],
                                         rhs=xt8[:, t, :], start=True, stop=False)
                        nc.tensor.matmul(out=prz[:, GT:2 * GT], lhsT=whrz_s[:, 64:128],
                                         rhs=hT[:], start=False, stop=True)
                        pin = gp2.tile([64, GT], F32, tag="pin")
                        nc.tensor.matmul(out=pin[:], lhsT=wihT_s[:, 128:192],
                                         rhs=xt8[:, t, :], start=True, stop=True)
                        phn = gp3.tile([64, GT], F32, tag="phn")
                        nc.tensor.matmul(out=phn[:], lhsT=whn_s[:], rhs=hT[:],
                                         start=True, stop=True)
                        rz = gv.tile([64, 2 * GT], BF16, tag="rz")
                        nc.scalar.activation(out=rz[:], in_=prz[:], func=AF.Sigmoid)
                        tmp = gv.tile([64, GT], BF16, tag="tmp")
                        nc.vector.tensor_mul(out=tmp[:], in0=rz[:, 0:GT], in1=phn[:])
                        t3 = gv.tile([64, GT], F32, tag="t3")
                        nc.vector.tensor_add(out=t3[:], in0=pin[:], in1=tmp[:])
                        nh = gv.tile([64, GT], BF16, tag="nh")
                        nc.scalar.activation(out=nh[:], in_=t3[:], func=AF.Tanh,
                                             bias=bihn_s[:, 0:1])
                        s1 = gv.tile([64, GT], BF16, tag="s1")
                        nc.vector.tensor_sub(out=s1[:], in0=hT[0:64, :], in1=nh[:])
                        s2 = gv.tile([64, GT], BF16, tag="s2")
                        nc.vector.tensor_mul(out=s2[:], in0=rz[:, GT:2 * GT], in1=s1[:])
                        nc.vector.tensor_add(out=hT[0:64, :], in0=nh[:], in1=s2[:])
                    for cc in range(GT // 128):
                        w = it * (GT // 128) + cc
                        pt = gp4.tile([128, 64], BF16, tag="pt")
                        nc.tensor.transpose(out=pt[:],
                                            in_=hT[0:64, cc * 128:(cc + 1) * 128],
                                            identity=identb[0:64, 0:64])
                        pat = gp4.tile([128, 8], F32, tag="pat")
                        nc.tensor.matmul(out=pat[:], lhsT=hT[0:64, cc * 128:(cc + 1) * 128],
                                         rhs=BC1_s[:], start=True, stop=True)
                        xc = gv.tile([128, 128], BF16, tag="xc")
                        nc.scalar.copy(out=xc[:, 0:64], in_=pt[:])
                        nc.vector.tensor_copy(out=xc[:, 64:68], in_=pat[:, 0:4])
                        nc.vector.memset(xc[:, 68:128], 0.0)
                        nc.vector.tensor_copy(out=adt1_sb[:, w, :], in_=pat[:, 4:8])
                        nc.sync.dma_start(out=xcat1l[w * 128:(w + 1) * 128, :], in_=xc[:])

            # ---- AllGather table1, split into shards ----
            tc.strict_bb_all_engine_barrier()
            nc.gpsimd.collective_compute(
                "AllGather", OP.bypass, replica_groups=rg_all,
                ins=[xcat1l[:]], outs=[table1[:]])
            for s in range(NSHARD):
                nc.sync.dma_start(out=tb1s[s][:], in_=table1[s * SH:(s + 1) * SH, :])
            tc.strict_bb_all_engine_barrier()

            # ---- GAT layer over windows (shared for layer 1 / layer 2) ----
            def gat_windows(tbls, adt_sb, nheads, payw, post_fn):
                """payw: scatter matmul width (4+256 for L1, 1+64 for L2)."""
                with (
                    tc.tile_pool(name="pg", bufs=3) as pg,
                    tc.tile_pool(name="pS", bufs=2) as pS,
                    tc.tile_pool(name="pd", bufs=2) as pd,
                    tc.tile_pool(name="pu", bufs=2) as pu,
                    tc.tile_pool(name="pM", bufs=2) as pM,
                    tc.tile_pool(name="pw", bufs=2) as pw,
                    tc.tile_pool(name="ps2", bufs=2, space="PSUM") as ps2,
                    tc.tile_pool(name="psA", bufs=1, space="PSUM") as psA,
                    tc.tile_pool(name="psB", bufs=1, space="PSUM") as psB,
                ):
                    for w in range(NW):
                        pblk = ps2.tile([128, payw], F32, tag="pblk")
                        first = True
                        for s in range(NSHARD):
                            b = w * NSHARD + s
                            Tb = int(tiles[b])
                            t0 = int(tile_off[b])
                            g = pg.tile([128, Tb, 128], BF16, tag=f"g{Tb}")
                            for q0 in range(0, Tb, 8):
                                qn = min(8, Tb - q0)
                                nc.gpsimd.dma_gather(
                                    out_ap=g[:, q0:q0 + qn, :], in_ap=tbls[s][:],
                                    idxs_ap=idx_sb[:, (t0 + q0) * 8:(t0 + q0 + qn) * 8],
                                    num_idxs=qn * 128, num_idxs_reg=nidx_regs[qn * 128],
                                    elem_size=128)
                            S = pS.tile([128, Tb, 128], BF16, tag=f"S{Tb}")
                            nc.vector.tensor_tensor(
                                out=S[:],
                                in0=iota_row[:].unsqueeze(1).broadcast_to([128, Tb, 128]),
                                in1=drel_sb[:, t0:t0 + Tb].unsqueeze(2).broadcast_to([128, Tb, 128]),
                                op=OP.is_equal)
                            drT = pd.tile([128, Tb, 128], U8, tag=f"dT{Tb}")
                            nc.sync.dma_start(
                                out=drT[:],
                                in_=drelT[0:1, t0:t0 + Tb, :].partition_broadcast(128))
                            Sd = pS.tile([128, Tb, 128], BF16, tag=f"Sd{Tb}")
                            nc.vector.tensor_scalar(
                                out=Sd[:], in0=drT[:], scalar1=iota_p[:, 0:1],
                                scalar2=None, op0=OP.is_equal)
                            padp = psA.tile([128, Tb * nheads], F32, tag="padp")
                            for t in range(Tb):
                                nc.tensor.matmul(
                                    out=padp[:, t * nheads:(t + 1) * nheads],
                                    lhsT=Sd[:, t, :], rhs=adt_sb[:, w, :],
                                    start=True, stop=True)
                            u = pu.tile([128, Tb, nheads], F32, tag=f"u{Tb}")
                            nc.vector.tensor_add(
                                out=u[:], in0=g[:, :, 64:64 + nheads],
                                in1=padp[:].rearrange("p (t c) -> p t c", t=Tb))
                            e1 = pu.tile([128, Tb, nheads], BF16, tag=f"e1{Tb}")
                            nc.scalar.activation(out=e1[:], in_=u[:], func=AF.Exp)
                            e2 = pu.tile([128, Tb, nheads], BF16, tag=f"e2{Tb}")
                            nc.scalar.activation(out=e2[:], in_=u[:], func=AF.Exp, scale=0.2)
                            ee = pu.tile([128, Tb, nheads], BF16, tag=f"ee{Tb}")
                            nc.vector.tensor_tensor(out=ee[:], in0=e1[:], in1=e2[:], op=OP.max)
                            M = pM.tile([128, Tb, payw], BF16, tag=f"M{Tb}")
                            nc.vector.tensor_copy(out=M[:, :, 0:nheads], in_=ee[:])
                            if nheads == 4:
                                nc.vector.tensor_tensor(
                                    out=M[:, :, 4:260].rearrange("p t (h c) -> p t h c", h=4),
                                    in0=g[:, :, 0:64].unsqueeze(2).broadcast_to([128, Tb, 4, 64]),
                                    in1=ee[:].unsqueeze(3).broadcast_to([128, Tb, 4, 64]),
                                    op=OP.mult)
                            else:
                                nc.vector.tensor_tensor(
                                    out=M[:, :, 1:65],
                                    in0=g[:, :, 0:64],
                                    in1=ee[:].to_broadcast([128, Tb, 64]),
                                    op=OP.mult)
                            for t in range(Tb):
                                nc.tensor.matmul(
                                    out=pblk[:], lhsT=S[:, t, :], rhs=M[:, t, :],
                                    start=first, stop=(s == NSHARD - 1 and t == Tb - 1))
                                first = False
                        post_fn(w, pblk, pw, psB)

            # ---- layer 1 post: h1 = relu(z/denom @ W1 + b1) -> xcat2, adt2 ----
            def post1(w, pblk, pw, psB):
                dn = pw.tile([128, 4], F32, tag="dn")
                nc.vector.tensor_scalar(out=dn[:], in0=pblk[:, 0:4], scalar1=1e-12,
                                        scalar2=None, op0=OP.max)
                rec = pw.tile([128, 4], F32, tag="rec")
                nc.vector.reciprocal(out=rec[:], in_=dn[:])
                zn = pw.tile([128, 256], BF16, tag="zn")
                nc.vector.tensor_tensor(
                    out=zn[:].rearrange("p (h c) -> p h c", h=4),
                    in0=pblk[:, 4:260].rearrange("p (h c) -> p h c", h=4),
                    in1=rec[:].unsqueeze(2).broadcast_to([128, 4, 64]),
                    op=OP.mult)
                tpz = pw.tile([128, 256], BF16, tag="tpz")
                for half in range(2):
                    ptp = psB.tile([128, 128], BF16, tag="ptp")
                    nc.tensor.transpose(out=ptp[:], in_=zn[:, 128 * half:128 * (half + 1)],
                                        identity=identb[:])
                    nc.scalar.copy(out=tpz[:, 128 * half:128 * (half + 1)], in_=ptp[:])
                h1p = psB.tile([128, 256], F32, tag="h1p")
                nc.tensor.matmul(out=h1p[:, 0:128], lhsT=tpz[:, 0:128], rhs=W12A_s[:],
                                 start=True, stop=True)
                nc.tensor.matmul(out=h1p[:, 128:256], lhsT=tpz[:, 128:256], rhs=W12B_s[:],
                                 start=True, stop=True)
                h1 = pw.tile([128, 256], BF16, tag="h1")
                nc.vector.tensor_add(out=h1[:], in0=h1p[:], in1=b1b_s[:])
                nc.vector.tensor_scalar(out=h1[:], in0=h1[:], scalar1=0.0,
                                        scalar2=None, op0=OP.max)
                th1 = pw.tile([128, 256], BF16, tag="th1")
                for half in range(2):
                    ptp = psB.tile([128, 128], BF16, tag="ptp")
                    nc.tensor.transpose(out=ptp[:], in_=h1[:, 128 * half:128 * (half + 1)],
                                        identity=identb[:])
                    nc.scalar.copy(out=th1[:, 128 * half:128 * (half + 1)], in_=ptp[:])
                xsc = psB.tile([128, 66], F32, tag="xsc")
                xs2p = xsc[:, 0:64]
                pat2 = xsc[:, 64:66]
                for half in range(2):
                    nc.tensor.matmul(out=xs2p, lhsT=th1[:, 128 * half:128 * (half + 1)],
                                     rhs=W2s_s[:, 64 * half:64 * (half + 1)],
                                     start=(half == 0), stop=(half == 1))
                    nc.tensor.matmul(out=pat2, lhsT=th1[:, 128 * half:128 * (half + 1)],
                                     rhs=BC2_s[:, 2 * half:2 * half + 2],
                                     start=(half == 0), stop=(half == 1))
                xc2 = pw.tile([128, 128], BF16, tag="xc2")
                nc.scalar.copy(out=xc2[:, 0:64], in_=xs2p)
                nc.vector.tensor_copy(out=xc2[:, 64:65], in_=pat2[:, 0:1])
                nc.vector.memset(xc2[:, 65:128], 0.0)
                nc.vector.tensor_copy(out=adt2_sb[:, w, :], in_=pat2[:, 1:2])
                nc.sync.dma_start(out=xcat2l[w * 128:(w + 1) * 128, :], in_=xc2[:])

            gat_windows(tb1s, adt1_sb, 4, 260, post1)

            tc.strict_bb_all_engine_barrier()
            nc.gpsimd.collective_compute(
                "AllGather", OP.bypass, replica_groups=rg_all,
                ins=[xcat2l[:]], outs=[table2[:]])
            for s in range(NSHARD):
                nc.sync.dma_start(out=tb2s[s][:], in_=table2[s * SH:(s + 1) * SH, :])
            tc.strict_bb_all_engine_barrier()

            # ---- layer 2 post: pooling into ppool psum ----
            pp_ctx = tc.tile_pool(name="psPool", bufs=1, space="PSUM")
            psPool = pp_ctx.__enter__()
            ppool = psPool.tile([128, 65], F32)

            def post2(w, pblk, pw, psB):
                dn2 = pw.tile([128, 1], F32, tag="dn2")
                nc.vector.tensor_scalar(out=dn2[:], in0=pblk[:, 0:1], scalar1=1e-12,
                                        scalar2=None, op0=OP.max)
                rec2 = pw.tile([128, 1], F32, tag="rec2")
                nc.vector.reciprocal(out=rec2[:], in_=dn2[:])
                ph = pw.tile([128, 65], BF16, tag="ph")
                nc.vector.tensor_scalar(out=ph[:, 0:64], in0=pblk[:, 1:65],
                                        scalar1=rec2[:, 0:1], scalar2=None, op0=OP.mult)
                nc.vector.tensor_add(out=ph[:, 0:64], in0=ph[:, 0:64], in1=b2b_s[:])
                nc.vector.tensor_scalar(out=ph[:, 0:64], in0=ph[:, 0:64], scalar1=0.0,
                                        scalar2=None, op0=OP.max)
                nc.vector.memset(ph[:, 64:65], 1.0)
                Sb = pw.tile([128, 128], BF16, tag="Sb")
                nc.vector.tensor_scalar(out=Sb[:], in0=iota_row[:],
                                        scalar1=batch_sb[:, w:w + 1], scalar2=None,
                                        op0=OP.is_equal)
                nc.tensor.matmul(out=ppool[:], lhsT=Sb[:], rhs=ph[:],
                                 start=(w == 0), stop=(w == NW - 1))

            gat_windows(tb2s, adt2_sb, 1, 65, post2)

            # ---- tail: AllReduce pools, fc, log_softmax ----
            with tc.tile_pool(name="rpre", bufs=1) as rpre:
                pr = rpre.tile([128, 65], F32)
                nc.scalar.copy(out=pr[:], in_=ppool[:])
                nc.sync.dma_start(out=arin[:], in_=pr[:])
            pp_ctx.__exit__(None, None, None)
            with (
                tc.tile_pool(name="r5", bufs=1) as r5,
                tc.tile_pool(name="r5q", bufs=1, space="PSUM") as r5q,
            ):
                tc.strict_bb_all_engine_barrier()
                nc.gpsimd.collective_compute(
                    "AllReduce", OP.add, replica_groups=rg_all,
                    ins=[arin[:]], outs=[arout[:]])
                tc.strict_bb_all_engine_barrier()
                ar = r5.tile([128, 65], F32)
                nc.sync.dma_start(out=ar[:], in_=arout[:])
                cm = r5.tile([128, 1], F32)
                nc.vector.tensor_scalar(out=cm[:], in0=ar[:, 64:65], scalar1=1.0,
                                        scalar2=None, op0=OP.max)
                cr = r5.tile([128, 1], F32)
                nc.vector.reciprocal(out=cr[:], in_=cm[:])
                gf = r5.tile([128, 64], F32)
                nc.vector.tensor_scalar(out=gf[:], in0=ar[:, 0:64], scalar1=cr[:, 0:1],
                                        scalar2=None, op0=OP.mult)
                identf = r5.tile([128, 128], F32)
                make_identity(nc, identf[:])
                pgt = r5q.tile([64, 128], F32)
                nc.tensor.transpose(out=pgt[:], in_=gf[:], identity=identf[:])
                gfT = r5.tile([64, 128], F32)
                nc.scalar.copy(out=gfT[:], in_=pgt[:])
                plg = r5q.tile([128, 10], F32)
                nc.tensor.matmul(out=plg[:], lhsT=gfT[:], rhs=fcw_s[:], start=True, stop=True)
                lg = r5.tile([128, 16], F32)
                nc.vector.tensor_add(out=lg[:, 0:10], in0=plg[:], in1=fcbb_s[:])
                mx = r5.tile([128, 1], F32)
                nc.vector.reduce_max(out=mx[:], in_=lg[:, 0:10], axis=mybir.AxisListType.X)
                tsh = r5.tile([128, 16], F32)
                nc.vector.tensor_scalar(out=tsh[:, 0:10], in0=lg[:, 0:10],
                                        scalar1=mx[:, 0:1], scalar2=None, op0=OP.subtract)
                exs = r5.tile([128, 16], F32)
                se = r5.tile([128, 1], F32)
                nc.scalar.activation(out=exs[:, 0:10], in_=tsh[:, 0:10], func=AF.Exp,
                                     accum_out=se[:])
                ln = r5.tile([128, 1], F32)
                nc.scalar.activation(out=ln[:], in_=se[:], func=AF.Ln)
                res = r5.tile([128, 16], F32)
                nc.vector.memset(res[:], 0.0)
                nc.vector.tensor_scalar(out=res[:, 0:10], in0=tsh[:, 0:10],
                                        scalar1=ln[:, 0:1], scalar2=None, op0=OP.subtract)
                nc.sync.dma_start(out=out[:], in_=res[:])
    nc.compile()
    return nc


# ---------------- self-contained entry point ----------------
_CACHE = {}


def kernel(**inputs):
    """Full DAGNN forward. Takes the unsharded inputs from setup_inputs();
    returns log-softmax output [num_graphs, 10] float32."""
    x = np.asarray(inputs["x"], np.float32)
    edge_index = np.asarray(inputs["edge_index"])
    batch = np.asarray(inputs["batch"])
    G = int(inputs["num_graphs"])
    weights = [np.asarray(inputs[k], np.float32) for k in (
        "gru_w_ih", "gru_w_hh", "gru_b_ih", "gru_b_hh",
        "W1", "att_src1", "att_dst1", "b1",
        "W2", "att_src2", "att_dst2", "b2", "fc_w", "fc_b")]
    N = x.shape[0]
    E = edge_index.shape[1]
    P = 8

    from concourse.bass_utils import run_bass_kernel_spmd
    cfg = Cfg(N, E, G, P)
    per_core = host_prep(cfg, edge_index, batch)
    in_maps = build_inputs(cfg, x, weights, per_core)
    key = (N, E, G, P, cfg.NPAD2, cfg.TOT_TILES, tuple(cfg.tiles[:8]))
    if key not in _CACHE:
        _CACHE[key] = build_kernel(cfg)
    nc = _CACHE[key]
    res = run_bass_kernel_spmd(nc, in_maps, core_ids=list(range(P)))
    out = np.asarray(res.results[0]["out"][:G, :10], np.float32)
    return out
